# revision 1
# baseline (speedup 1.0000x reference)
"""Causal self-attention (B=2, S=2048, E=1024, H=16) on 8 TRN2 NeuronCores.

Sharding: core c = 4*b + g handles batch b and head-group g (4 heads,
256 E-columns). Each core computes q/k/v projections for its head slice,
causal attention for its 4 heads, and a partial output projection
y_c = ctx_g @ Wo[rows_g]. Host sums the 4 partials per batch and adds bo.

Engine plan (CoreSim cost model driven; ~100us/core vs 172.6us baseline):
  PE   - QKV projections in fp8e4 DoubleRow (K=256/instr, 0.5 cyc/row) with
         3-term error compensation (x8@W8 + x8@rW8 + rx8@W8) accumulated in
         one PSUM group; scores in fp8 DoubleRow with the head dim folded
         32x2 onto partitions (q/k re-quantized to fp8; half-rate scores);
         A@V in natural orientation (out = [128 q-parts, 65] bf16, the
         denominator rides as V's ones column; all 4 heads of a stile share
         one PSUM bank via per-byte zero-region semantics + a nosync order
         edge on the single start=True matmul); out-proj in bf16 from a
         folded ctxT.
  ACT  - exp only (the floor: ~8.4M exps/core = 72.8us; the emission order
         makes exp the continuously-fed pacer; chunks processed 0,1,3,2 so
         the smallest remaining tail follows the last exp).
  DVE  - q/k bias+descale (psum->fp8), v bias+descale, denominators'
         reciprocal + ctx normalize, y psum->sbuf (GPSIMD cannot touch
         PSUM on real HW - walrus birverifier enforces this).
  Pool - causal mask multiply on te (sbuf only), small-constant DMAs.
  DMA  - fp8/bf16 inputs, q/k fp8 fold via partition-shift SBUF DMAs,
         XBAR dma_start_transpose builds ctxT off-engine, bf16 y partials.

Emission = Tile scheduler priority: [all scores/exp chains, chunk-ordered,
with the next wave's q/k proj between chunks] then [v proj + ctx + out-proj
as hole-fillers] - deferrable PE work migrates into exp-paced windows.

PSUM banks (8): scores ring [128,2,512]x2 = 4 (bank-per-head-pair),
ctx/v ring [128,512]x2 = 2 (4 heads per bank), qk-proj/out-proj ring
[128,512]x2 = 2.
"""

import os

import numpy as np
import ml_dtypes

os.environ.setdefault("NEURON_RT_RESET_CORES", "1")

B, S, E, H, D = 2, 2048, 1024, 16, 64
NCORES = 8
EC = 256          # E-columns per core (4 heads x 64)
QC = 512          # q-chunk width
NQC = S // QC     # 4
NKT = S // 128    # 16 k-tiles
NKS = 4           # fp8 DoubleRow K-steps (4 x 256 = 1024)

WSQ = 256.0       # fp8 range scale for Wq*(1/8)
WSK = 64.0        # fp8 range scale for Wk / Wv
E4NP = ml_dtypes.float8_e4m3
BFNP = ml_dtypes.bfloat16

_CACHE = {}


def _build_nc(cfg=None):
    cfg = cfg or {}
    import concourse.mybir as mybir
    import concourse.tile as tile
    import concourse.bass as bass
    from concourse import bacc

    F32 = mybir.dt.float32
    BF16 = mybir.dt.bfloat16
    F8 = mybir.dt.float8e4
    EXP = mybir.ActivationFunctionType.Exp
    DR = mybir.MatmulPerfMode.DoubleRow
    MUL = mybir.AluOpType.mult
    ADD = mybir.AluOpType.add

    TE_BUFS = cfg.get("te", 44)
    nc = bacc.Bacc("TRN2", target_bir_lowering=False, debug=False)

    x8 = nc.dram_tensor("x8", [128, NKS, 2, S], F8, kind="ExternalInput")
    rx8 = nc.dram_tensor("rx8", [128, NKS, 2, S], F8, kind="ExternalInput")
    wq8 = nc.dram_tensor("wq8", [128, NKS, 2, EC], F8, kind="ExternalInput")
    wk8 = nc.dram_tensor("wk8", [128, NKS, 2, EC], F8, kind="ExternalInput")
    wv8 = nc.dram_tensor("wv8", [128, NKS, 2, EC], F8, kind="ExternalInput")
    rwq8 = nc.dram_tensor("rwq8", [128, NKS, 2, EC], F8, kind="ExternalInput")
    rwk8 = nc.dram_tensor("rwk8", [128, NKS, 2, EC], F8, kind="ExternalInput")
    rwv8 = nc.dram_tensor("rwv8", [128, NKS, 2, EC], F8, kind="ExternalInput")
    wo = nc.dram_tensor("wo", [128, 2, E], BF16, kind="ExternalInput")
    bq = nc.dram_tensor("bq", [2, 128, 1], F32, kind="ExternalInput")
    bk = nc.dram_tensor("bk", [2, 128, 1], F32, kind="ExternalInput")
    bv = nc.dram_tensor("bv", [1, EC], F32, kind="ExternalInput")
    msk = nc.dram_tensor("msk", [128, 128], BF16, kind="ExternalInput")
    ones = nc.dram_tensor("ones", [1, 64], BF16, kind="ExternalInput")

    y = nc.dram_tensor("y", [S, E], BF16, kind="ExternalOutput")

    with tile.TileContext(nc) as tc:
        with (
            tc.tile_pool(name="weights", bufs=1) as wpool,
            tc.tile_pool(name="xp", bufs=1) as xp,
            tc.tile_pool(name="qkv", bufs=1) as qkv,
            tc.tile_pool(name="tep", bufs=TE_BUFS) as tep,
            tc.tile_pool(name="tcnp", bufs=8) as tcnp,
            tc.tile_pool(name="rcp", bufs=4) as rcp,
            tc.tile_pool(name="ctp", bufs=1) as ctp,
            tc.tile_pool(name="typ", bufs=10) as typ,
            tc.tile_pool(name="smalls", bufs=1) as smalls,
            tc.tile_pool(name="scp", bufs=2, space="PSUM") as scp,
            tc.tile_pool(name="cxp", bufs=2, space="PSUM") as cxp,
            tc.tile_pool(name="prp", bufs=2, space="PSUM") as prp,
        ):
            # ---- small constants (Pool SWDGE queue) ----
            tbq = smalls.tile([128, 2], F32, tag="bq")
            tbk = smalls.tile([128, 2], F32, tag="bk")
            tbv = smalls.tile([128, EC], F32, tag="bv")
            tmsk = smalls.tile([128, 128], BF16, tag="msk")

            for r in range(2):
                nc.gpsimd.dma_start(tbq[:, r:r + 1], bq[r])
                nc.gpsimd.dma_start(tbk[:, r:r + 1], bk[r])
            bvap = bv[0, :]
            bv_b = bass.AP(tensor=bvap.tensor, offset=bvap.offset,
                           ap=[[0, 128]] + list(bvap.ap))
            nc.gpsimd.dma_start(tbv[:], bv_b)
            nc.gpsimd.dma_start(tmsk[:], msk[:])

            # ---- x fp8 (+ residual), loaded per 512-wide wave ----
            tx = xp.tile([128, NKS, 2, S], F8, tag="x8")
            trx = xp.tile([128, NKS, 2, S], F8, tag="rx8")

            def ldx(w):
                sl = slice(w * QC, (w + 1) * QC)
                nc.gpsimd.dma_start(tx[:, :, :, sl], x8[:, :, :, sl])
                nc.gpsimd.dma_start(trx[:, :, :, sl], rx8[:, :, :, sl])

            # wave-0 x + q/k weights first so the q/k chain starts ASAP
            sl0 = slice(0, QC)
            tw = {}

            def ldw(nm, dram):
                t = wpool.tile([128, NKS, 2, EC], F8, tag=nm, name=nm)
                nc.sync.dma_start(t[:], dram[:])
                tw[nm] = t

            ldw("wq", wq8)
            ldw("wk", wk8)
            for ks in range(NKS):
                nc.scalar.dma_start(tx[:, ks, :, sl0], x8[:, ks, :, sl0])
            ldw("rwq", rwq8)
            ldw("rwk", rwk8)
            nc.scalar.dma_start(trx[:, :, :, sl0], rx8[:, :, :, sl0])
            ldw("wv", wv8)
            ldw("rwv", rwv8)
            two = wpool.tile([128, 2, E], BF16, tag="wo")
            nc.sync.dma_start(two[:], wo[:])
            for w_ in range(1, NQC):
                ldx(w_)

            # ---- persistent activations ----
            SCF8 = cfg.get("sc_fp8", True)
            QKDT = F8 if SCF8 else BF16
            # natural layout (partition = feature within r-tile)
            tq = [qkv.tile([128, S], QKDT, tag=f"q{r}", name=f"q{r}")
                  for r in range(2)]
            tk = [qkv.tile([128, S], QKDT, tag=f"k{r}", name=f"k{r}")
                  for r in range(2)]
            # folded fp8 layout: partition = head_local*32 + d_low,
            # slot = d_high half; built by partition-shift DMAs
            if SCF8:
                tqf = qkv.tile([128, 2, S], F8, tag="qf", name="qf")
                tkf = qkv.tile([128, 2, S], F8, tag="kf", name="kf")
            # v1: [128, kt, head, 65]; col 64 of each head block = 1.0
            tv = qkv.tile([128, NKT, 4, 65], BF16, tag="v")
            onesap = ones[0, 0:1]
            ones_v = bass.AP(tensor=onesap.tensor, offset=onesap.offset,
                             ap=[[0, 128], [0, NKT * 4], [0, 1]])
            nc.gpsimd.dma_start(tv[:, :, :, 64:65], ones_v)

            tct = ctp.tile([128, 2, S], BF16, tag="ct")  # folded ctxT

            def mask_b(n):
                m = tmsk[:]
                return bass.AP(tensor=m.tensor, offset=m.offset,
                               ap=[list(m.ap[0]), [0, 2], [1, n]])

            def colb(ap1, n):
                # [128, 1] AP -> [128, n] stride-0 broadcast
                return bass.AP(tensor=ap1.tensor, offset=ap1.offset,
                               ap=[list(ap1.ap[0]), [0, n]])

            def colb2(ap2, n):
                # [128, k] AP -> [128, k, n] stride-0 broadcast
                return bass.AP(tensor=ap2.tensor, offset=ap2.offset,
                               ap=[list(ap2.ap[0]), list(ap2.ap[1]), [0, n]])

            def apx(t, off, dims):
                # partition dim of tile t + custom free dims at f32-col offset
                a = t[:]
                return bass.AP(tensor=a.tensor, offset=a.offset + off,
                               ap=[list(a.ap[0])] + [list(d) for d in dims])

            def order(later, first):
                bass._add_dep_helper(later.ins, first.ins, sync=False,
                                     reason="psum zero-region order")

            COMP = ((None, tx), ("r", tx), (None, trx))  # (w-residual?, x-tensor)

            def qk_unit(w, r, wn):
                dst, ws, bias = ((tq, WSQ, tbq) if wn == "wq"
                                 else (tk, WSK, tbk))
                sl = slice(w * QC, (w + 1) * QC)
                prt = prp.tile([128, QC], F32, tag="pr", name=f"p{wn}{w}_{r}")
                idx = 0
                for res, xt in COMP:
                    wt = tw[("r" if res else "") + wn]
                    for ks in range(NKS):
                        nc.tensor.matmul(
                            prt[:],
                            wt[:, ks, :, r * 128:(r + 1) * 128],
                            xt[:, ks, :, sl],
                            start=(idx == 0), stop=(idx == 3 * NKS - 1),
                            perf_mode=DR)
                        idx += 1
                nc.vector.scalar_tensor_tensor(
                    dst[r][:, sl], prt[:], 1.0 / ws,
                    colb(bias[:, r:r + 1], QC), op0=MUL, op1=ADD)
                if SCF8:
                    fdst = tqf if wn == "wq" else tkf
                    for h2 in range(2):
                        for slot in range(2):
                            nc.sync.dma_start(
                                fdst[(2 * r + h2) * 32:(2 * r + h2) * 32 + 32,
                                     slot, sl],
                                dst[r][h2 * 64 + slot * 32:
                                       h2 * 64 + slot * 32 + 32, sl])

            def v_unit(w, pair):
                cxt = cxp.tile([128, 512], F32, tag="cx", name=f"pv{w}_{pair}")
                first = None
                for st2 in range(2):
                    st = 4 * w + 2 * pair + st2
                    idx = 0
                    for res, xt in COMP:
                        wt = tw[("r" if res else "") + "wv"]
                        for ks in range(NKS):
                            m = nc.tensor.matmul(
                                cxt[:, st2 * EC:(st2 + 1) * EC],
                                xt[:, ks, :, st * 128:(st + 1) * 128],
                                wt[:, ks, :, :],
                                start=(first is None), stop=False,
                                perf_mode=DR, skip_group_check=True)
                            if first is None:
                                first = m
                            elif idx == 0:
                                order(m, first)
                            idx += 1
                for st2 in range(2):
                    st = 4 * w + 2 * pair + st2
                    nc.vector.scalar_tensor_tensor(
                        tv[:, st, :, 0:64],
                        apx(cxt, st2 * EC, [[64, 4], [1, 64]]),
                        1.0 / WSK,
                        tbv[:].rearrange("p (h d) -> p h d", h=4),
                        op0=MUL, op1=ADD)

            def proj_wave(w):
                for r in range(2):
                    qk_unit(w, r, "wq")
                    qk_unit(w, r, "wk")
                for p in range(2):
                    v_unit(w, p)

            all_tes = {}

            def scores_phase(qc):
                n_kt = 4 * (qc + 1)
                tes = [[], []]
                all_tes[qc] = tes
                for hp in range(2):
                    for kt in range(n_kt):
                        dg = kt - 4 * qc
                        coff = 128 * dg if dg > 0 else 0
                        sct = scp.tile([128, 2, QC], F32, tag="sc",
                                       name=f"s{qc}_{hp}_{kt}")
                        for h2 in range(2):
                            if SCF8:
                                hl = (2 * hp + h2) * 32
                                nc.tensor.matmul(
                                    sct[:, h2, coff:QC],
                                    tkf[hl:hl + 32, :,
                                        kt * 128:(kt + 1) * 128],
                                    tqf[hl:hl + 32, :,
                                        qc * QC + coff:(qc + 1) * QC],
                                    start=True, stop=True, perf_mode=DR,
                                    tile_position=(hl, 0))
                            else:
                                bp = h2 * 64
                                nc.tensor.matmul(
                                    sct[:, h2, coff:QC],
                                    tk[hp][bp:bp + 64, kt * 128:(kt + 1) * 128],
                                    tq[hp][bp:bp + 64,
                                           qc * QC + coff:(qc + 1) * QC],
                                    start=True, stop=True)
                        tet = tep.tile([128, 2, QC], BF16, tag="te",
                                       name=f"te{qc}_{hp}_{kt}")
                        nc.scalar.activation(
                            tet[:, :, coff:QC], sct[:, :, coff:QC], EXP)
                        if dg >= 0:
                            nc.gpsimd.tensor_mul(
                                tet[:, :, coff:coff + 128],
                                tet[:, :, coff:coff + 128],
                                mask_b(128))
                        tes[hp].append(tet)

            def ctx_phase(qc, op_inline=False):
                tes = all_tes[qc]
                for sti in range(4):
                    st = 4 * qc + sti
                    cxt = cxp.tile([128, 512], F32, tag="cx",
                                   name=f"cx{qc}_{sti}")
                    first = None
                    for hp in range(2):
                        for h2 in range(2):
                            h = 2 * hp + h2
                            for kt in range(st + 1):
                                m = nc.tensor.matmul(
                                    cxt[:, h * 128:h * 128 + 65],
                                    tes[hp][kt][:, h2,
                                                sti * 128:(sti + 1) * 128],
                                    tv[:, kt, h, :],
                                    start=(first is None), stop=False,
                                    skip_group_check=True)
                                if first is None:
                                    first = m
                                elif kt == 0:
                                    order(m, first)
                    rc = rcp.tile([128, 4], F32, tag="rc",
                                  name=f"rc{qc}_{sti}")
                    nc.vector.reciprocal(rc[:], apx(cxt, 64, [[128, 4], [1, 1]]))
                    tcn = tcnp.tile([128, 4, 64], BF16, tag="cn",
                                    name=f"cn{qc}_{sti}")
                    nc.vector.tensor_mul(
                        tcn[:], apx(cxt, 0, [[128, 4], [1, 64]]),
                        colb2(rc[:], 64))
                    for hp in range(2):
                        nc.sync.dma_start_transpose(
                            tct[:, hp, st * 128:(st + 1) * 128],
                            tcn[:, 2 * hp:2 * hp + 2, :])
                    if op_inline:
                        out_proj_sti(qc, sti)

            def out_proj_sti(qc, sti, act_copy=False):
                    st = 4 * qc + sti
                    for nn in range(2):
                        pyt = prp.tile([128, QC], F32, tag="pr",
                                       name=f"py{qc}_{sti}_{nn}")
                        for hp in range(2):
                            nc.tensor.matmul(
                                pyt[:],
                                tct[:, hp, st * 128:(st + 1) * 128],
                                two[:, hp, nn * QC:(nn + 1) * QC],
                                start=(hp == 0), stop=(hp == 1))
                        ty = typ.tile([128, QC], BF16, tag="y",
                                      name=f"y{qc}_{sti}_{nn}")
                        if act_copy:
                            nc.scalar.copy(ty[:], pyt[:])
                        else:
                            nc.vector.tensor_copy(ty[:], pyt[:])
                        nc.sync.dma_start(
                            y[st * 128:(st + 1) * 128,
                              nn * QC:(nn + 1) * QC], ty[:])

            def out_proj(qc, act_copy=False):
                for sti in range(4):
                    out_proj_sti(qc, sti, act_copy)

            # Emission = scheduler priority. The exp stream is the global
            # pacer: scores/exp chains first (chunk order) with the next
            # wave's q/k projection between them, then all deferrable work.
            CORDER = cfg.get("corder", [0, 1, 3, 2])
            # waves needed before a chunk's scores: all w <= qc
            emitted_qk = set()

            def need_qk(qc):
                for w_ in range(qc + 1):
                    if w_ not in emitted_qk:
                        emitted_qk.add(w_)
                        for r in range(2):
                            qk_unit(w_, r, "wq")
                            qk_unit(w_, r, "wk")

            need_qk(CORDER[0])
            for i, qc in enumerate(CORDER):
                scores_phase(qc)
                if i + 1 < NQC:
                    need_qk(CORDER[i + 1])
            emitted_v = set()

            def need_v(qc):
                for w_ in range(qc + 1):
                    if w_ not in emitted_v:
                        emitted_v.add(w_)
                        for p in range(2):
                            v_unit(w_, p)

            need_v(CORDER[0])
            for i, qc in enumerate(CORDER):
                if i + 1 < NQC:
                    need_v(CORDER[i + 1])
                ctx_phase(qc)
                out_proj(qc, act_copy=(i == NQC - 1))

    nc.compile()
    return nc


def _get_nc():
    if "nc" not in _CACHE:
        _CACHE["nc"] = _build_nc()
    return _CACHE["nc"]


def make_mask():
    kl = np.arange(128)[:, None]
    ql = np.arange(128)[None, :]
    return (ql >= kl).astype(BFNP)


def _fold(t):
    # [E, N] -> [128, NKS, 2, N] with e = ks*256 + sl*128 + p
    n = t.shape[1]
    return np.ascontiguousarray(
        t.reshape(NKS, 2, 128, n).transpose(2, 0, 1, 3))


def _q8(t):
    t8 = t.astype(E4NP)
    return t8, (t - t8.astype(np.float32)).astype(E4NP)


def shard_inputs(x, Wq, bq, Wk, bk, Wv, bv, Wo, bo):
    x = np.asarray(x, dtype=np.float32)
    scale = np.float32(1.0 / np.sqrt(D))
    mask = make_mask()
    ones = np.ones((1, 64), BFNP)
    in_maps = []
    xf = []
    for b in range(B):
        x8, rx8 = _q8(_fold(np.ascontiguousarray(x[b].T)))
        xf.append((x8, rx8))
    for c in range(NCORES):
        b, g = divmod(c, 4)
        cs = slice(g * EC, (g + 1) * EC)
        wq, rwq = _q8(_fold(np.asarray(Wq[:, cs]) * (scale * WSQ)))
        wk, rwk = _q8(_fold(np.asarray(Wk[:, cs]) * WSK))
        wv, rwv = _q8(_fold(np.asarray(Wv[:, cs]) * WSK))
        in_maps.append({
            "x8": xf[b][0], "rx8": xf[b][1],
            "wq8": wq, "rwq8": rwq,
            "wk8": wk, "rwk8": rwk,
            "wv8": wv, "rwv8": rwv,
            "wo": np.ascontiguousarray(
                np.asarray(Wo[cs, :]).reshape(2, 128, E).transpose(1, 0, 2)
            ).astype(BFNP),
            "bq": (np.asarray(bq[cs]) * scale).reshape(2, 128, 1).astype(np.float32),
            "bk": np.asarray(bk[cs]).reshape(2, 128, 1).astype(np.float32),
            "bv": np.asarray(bv[cs]).reshape(1, EC).astype(np.float32),
            "msk": mask,
            "ones": ones,
        })
    return in_maps


def combine_outputs(results, bo):
    y = np.zeros((B, S, E), np.float32)
    for c in range(NCORES):
        b = c // 4
        y[b] += np.asarray(results[c]["y"], dtype=np.float32)
    y += np.asarray(bo, dtype=np.float32)[None, None, :]
    return y


def kernel(x, Wq, bq, Wk, bk, Wv, bv, Wo, bo):
    from concourse.bass_utils import run_bass_kernel_spmd

    nc = _get_nc()
    in_maps = shard_inputs(x, Wq, bq, Wk, bk, Wv, bv, Wo, bo)
    try:
        res = run_bass_kernel_spmd(nc, in_maps, core_ids=list(range(NCORES)))
    except Exception:
        # transient device errors (e.g. a wedged core) usually clear on retry
        res = run_bass_kernel_spmd(nc, in_maps, core_ids=list(range(NCORES)))
    return combine_outputs(res.results, bo)



# revision 51
# speedup vs baseline: 1.0667x; 1.0667x over previous
"""Causal self-attention (B=2, S=2048, E=1024, H=16) on 8 TRN2 NeuronCores.

Sharding: core c = 4*b + g handles batch b and head-group g (4 heads,
256 E-columns). Each core computes q/k/v projections for its head slice,
causal attention for its 4 heads, and a partial output projection
y_c = ctx_g @ Wo[rows_g]. Host sums the 4 partials per batch and adds bo.

Engine plan (CoreSim cost model driven; ~100us/core vs 172.6us baseline):
  PE   - QKV projections in fp8e4 DoubleRow (K=256/instr, 0.5 cyc/row) with
         3-term error compensation (x8@W8 + x8@rW8 + rx8@W8) accumulated in
         one PSUM group; scores in fp8 DoubleRow with the head dim folded
         32x2 onto partitions (q/k re-quantized to fp8; half-rate scores);
         A@V in natural orientation (out = [128 q-parts, 65] bf16, the
         denominator rides as V's ones column; all 4 heads of a stile share
         one PSUM bank via per-byte zero-region semantics + a nosync order
         edge on the single start=True matmul); out-proj in bf16 from a
         folded ctxT.
  ACT  - exp only (the floor: ~8.4M exps/core = 72.8us; the emission order
         makes exp the continuously-fed pacer; chunks processed 0,1,3,2 so
         the smallest remaining tail follows the last exp).
  DVE  - q/k bias+descale (psum->fp8), v bias+descale, denominators'
         reciprocal + ctx normalize, y psum->sbuf (GPSIMD cannot touch
         PSUM on real HW - walrus birverifier enforces this).
  Pool - causal mask multiply on te (sbuf only), small-constant DMAs.
  DMA  - fp8/bf16 inputs, q/k fp8 fold via partition-shift SBUF DMAs,
         XBAR dma_start_transpose builds ctxT off-engine, bf16 y partials.

Emission = Tile scheduler priority: [all scores/exp chains, chunk-ordered,
with the next wave's q/k proj between chunks] then [v proj + ctx + out-proj
as hole-fillers] - deferrable PE work migrates into exp-paced windows.

PSUM banks (8): scores ring [128,2,512]x2 = 4 (bank-per-head-pair),
ctx/v ring [128,512]x2 = 2 (4 heads per bank), qk-proj/out-proj ring
[128,512]x2 = 2.
"""

import os

import numpy as np
import ml_dtypes

os.environ.setdefault("NEURON_RT_RESET_CORES", "1")

B, S, E, H, D = 2, 2048, 1024, 16, 64
NCORES = 8
EC = 256          # E-columns per core (4 heads x 64)
QC = 512          # q-chunk width
NQC = S // QC     # 4
NKT = S // 128    # 16 k-tiles
NKS = 4           # fp8 DoubleRow K-steps (4 x 256 = 1024)

WSQ = 256.0       # fp8 range scale for Wq*(1/8)
WSK = 64.0        # fp8 range scale for Wk / Wv
E4NP = ml_dtypes.float8_e4m3
BFNP = ml_dtypes.bfloat16

_CACHE = {}


def _build_nc(cfg=None):
    cfg = cfg or {}
    import concourse.mybir as mybir
    import concourse.tile as tile
    import concourse.bass as bass
    from concourse import bacc

    F32 = mybir.dt.float32
    BF16 = mybir.dt.bfloat16
    F8 = mybir.dt.float8e4
    EXP = mybir.ActivationFunctionType.Exp
    DR = mybir.MatmulPerfMode.DoubleRow
    MUL = mybir.AluOpType.mult
    ADD = mybir.AluOpType.add

    TE_BUFS = cfg.get("te", 44)
    nc = bacc.Bacc("TRN2", target_bir_lowering=False, debug=False)

    x8 = nc.dram_tensor("x8", [128, NKS, 2, S], F8, kind="ExternalInput")
    rx8 = nc.dram_tensor("rx8", [128, NKS, 2, S], F8, kind="ExternalInput")
    wq8 = nc.dram_tensor("wq8", [128, NKS, 2, EC], F8, kind="ExternalInput")
    wk8 = nc.dram_tensor("wk8", [128, NKS, 2, EC], F8, kind="ExternalInput")
    wv8 = nc.dram_tensor("wv8", [128, NKS, 2, EC], F8, kind="ExternalInput")
    rwq8 = nc.dram_tensor("rwq8", [128, NKS, 2, EC], F8, kind="ExternalInput")
    rwk8 = nc.dram_tensor("rwk8", [128, NKS, 2, EC], F8, kind="ExternalInput")
    rwv8 = nc.dram_tensor("rwv8", [128, NKS, 2, EC], F8, kind="ExternalInput")
    wo = nc.dram_tensor("wo", [128, 2, E], BF16, kind="ExternalInput")
    bq = nc.dram_tensor("bq", [2, 128, 1], F32, kind="ExternalInput")
    bk = nc.dram_tensor("bk", [2, 128, 1], F32, kind="ExternalInput")
    bv = nc.dram_tensor("bv", [1, EC], F32, kind="ExternalInput")
    msk = nc.dram_tensor("msk", [128, 128], BF16, kind="ExternalInput")
    imat = nc.dram_tensor("imat", [128, 128], BF16, kind="ExternalInput")
    ones = nc.dram_tensor("ones", [1, 64], BF16, kind="ExternalInput")

    y = nc.dram_tensor("y", [S, E], BF16, kind="ExternalOutput")

    with tile.TileContext(nc) as tc:
        with (
            tc.tile_pool(name="weights", bufs=1) as wpool,
            tc.tile_pool(name="xp", bufs=1) as xp,
            tc.tile_pool(name="qkv", bufs=1) as qkv,
            tc.tile_pool(name="tep", bufs=TE_BUFS) as tep,
            tc.tile_pool(name="tcnp", bufs=8) as tcnp,
            tc.tile_pool(name="rcp", bufs=4) as rcp,
            tc.tile_pool(name="ctp", bufs=1) as ctp,
            tc.tile_pool(name="typ", bufs=10) as typ,
            tc.tile_pool(name="smalls", bufs=1) as smalls,
            tc.tile_pool(name="scp", bufs=2, space="PSUM") as scp,
            tc.tile_pool(name="cxp", bufs=2, space="PSUM") as cxp,
            tc.tile_pool(name="prp", bufs=2, space="PSUM") as prp,
        ):
            # ---- small constants (Pool SWDGE queue) ----
            tbq = smalls.tile([128, 2], F32, tag="bq")
            tbk = smalls.tile([128, 2], F32, tag="bk")
            tbv = smalls.tile([128, EC], F32, tag="bv")
            tmsk = smalls.tile([128, 128], BF16, tag="msk")
            timat = smalls.tile([128, 128], BF16, tag="imat")

            for r in range(2):
                nc.gpsimd.dma_start(tbq[:, r:r + 1], bq[r])
                nc.gpsimd.dma_start(tbk[:, r:r + 1], bk[r])
            bvap = bv[0, :]
            bv_b = bass.AP(tensor=bvap.tensor, offset=bvap.offset,
                           ap=[[0, 128]] + list(bvap.ap))
            nc.gpsimd.dma_start(tbv[:], bv_b)
            nc.gpsimd.dma_start(tmsk[:], msk[:])
            nc.gpsimd.dma_start(timat[:], imat[:])

            # ---- x fp8 (+ residual), loaded per 512-wide wave ----
            tx = xp.tile([128, NKS, 2, S], F8, tag="x8")
            trx = xp.tile([128, NKS, 2, S], F8, tag="rx8")

            def ldx(w):
                sl = slice(w * QC, (w + 1) * QC)
                nc.gpsimd.dma_start(tx[:, :, :, sl], x8[:, :, :, sl])
                nc.gpsimd.dma_start(trx[:, :, :, sl], rx8[:, :, :, sl])

            # wave-0 x + q/k weights first so the q/k chain starts ASAP.
            # Weights dispatch on SP, wave-0 x/rx on DVE (both idle at start)
            # so neither queue serializes the first-exp critical path.
            sl0 = slice(0, QC)
            tw = {}

            def ldw(nm, dram, eng=None):
                t = wpool.tile([128, NKS, 2, EC], F8, tag=nm, name=nm)
                (eng or nc.sync).dma_start(t[:], dram[:])
                tw[nm] = t

            # startup: wave-0 x/rx arrive in 128-col quarters, spread over
            # the SP and ACT HWDGE queues so quarter 0 + the q/k weights
            # land as early as possible for the first quarter-projection
            def ldxq(eng, xt, dram, j):
                eng.dma_start(xt[:, :, :, j * 128:(j + 1) * 128],
                              dram[:, :, :, j * 128:(j + 1) * 128])

            ldxq(nc.sync, tx, x8, 0)
            ldw("wq", wq8)
            ldxq(nc.scalar, trx, rx8, 0)
            ldw("wk", wk8)
            ldw("rwk", rwk8, eng=nc.scalar)
            ldw("rwq", rwq8)
            for j in range(1, 4):
                ldxq(nc.scalar, trx, rx8, j)
            for j in range(1, 4):
                ldxq(nc.sync, tx, x8, j)
            # wave-1 x on SP so its q/k projection is ready the moment
            # chunk-0's exps drain (Pool's SWDGE queue is too slow for it)
            sl1 = slice(QC, 2 * QC)
            nc.sync.dma_start(tx[:, :, :, sl1], x8[:, :, :, sl1])
            nc.sync.dma_start(trx[:, :, :, sl1], rx8[:, :, :, sl1])
            ldw("wv", wv8)
            ldw("rwv", rwv8)
            two = wpool.tile([128, 2, E], BF16, tag="wo")
            nc.sync.dma_start(two[:], wo[:])
            for w_ in range(2, NQC):
                ldx(w_)

            # ---- persistent activations ----
            # folded fp8 layout: partition = head_local*32 + d_low,
            # slot = d_high half; written DIRECTLY by the projection
            # (weight columns are slot-major-permuted on the host), so no
            # partition-shift fold DMAs are needed.
            tqf = qkv.tile([128, 2, S], F8, tag="qf", name="qf")
            tkf = qkv.tile([128, 2, S], F8, tag="kf", name="kf")
            # v1: [128, kt, head, 65]; col 64 of each head block = 1.0
            tv = qkv.tile([128, NKT, 4, 65], BF16, tag="v")
            onesap = ones[0, 0:1]
            ones_v = bass.AP(tensor=onesap.tensor, offset=onesap.offset,
                             ap=[[0, 128], [0, NKT * 4], [0, 1]])
            nc.gpsimd.dma_start(tv[:, :, :, 64:65], ones_v)

            tct = ctp.tile([128, 2, S], BF16, tag="ct")  # folded ctxT

            def colb(ap1, n):
                # [128, 1] AP -> [128, n] stride-0 broadcast
                return bass.AP(tensor=ap1.tensor, offset=ap1.offset,
                               ap=[list(ap1.ap[0]), [0, n]])

            def colb2(ap2, n):
                # [128, k] AP -> [128, k, n] stride-0 broadcast
                return bass.AP(tensor=ap2.tensor, offset=ap2.offset,
                               ap=[list(ap2.ap[0]), list(ap2.ap[1]), [0, n]])

            def apx(t, off, dims):
                # partition dim of tile t + custom free dims at f32-col offset
                a = t[:]
                return bass.AP(tensor=a.tensor, offset=a.offset + off,
                               ap=[list(a.ap[0])] + [list(d) for d in dims])

            def order(later, first):
                bass._add_dep_helper(later.ins, first.ins, sync=False,
                                     reason="psum zero-region order")

            COMP = ((None, tx), ("r", tx), (None, trx))  # (w-residual?, x-tensor)
            COMP_QK = COMP[:2] if cfg.get("qk2") else COMP
            COMP_V = COMP[:2] if cfg.get("v2") else COMP

            def qk_unit(w, slot, wn, q4=None):
                fdst, ws, bias = ((tqf, WSQ, tbq) if wn == "wq"
                                  else (tkf, WSK, tbk))
                if q4 is None:
                    sl = slice(w * QC, (w + 1) * QC)
                    width = QC
                else:
                    sl = slice(w * QC + q4 * 128, w * QC + (q4 + 1) * 128)
                    width = 128
                prt = prp.tile([128, QC], F32, tag="pr",
                               name=f"p{wn}{w}_{slot}_{q4}")[:, 0:width]
                idx = 0
                for res, xt in COMP_QK:
                    wt = tw[("r" if res else "") + wn]
                    for ks in range(NKS):
                        nc.tensor.matmul(
                            prt,
                            wt[:, ks, :, slot * 128:(slot + 1) * 128],
                            xt[:, ks, :, sl],
                            start=(idx == 0),
                            stop=(idx == len(COMP_QK) * NKS - 1),
                            perf_mode=DR)
                        idx += 1
                nc.vector.scalar_tensor_tensor(
                    fdst[:, slot, sl], prt, 1.0 / ws,
                    colb(bias[:, slot:slot + 1], width), op0=MUL, op1=ADD)

            def v_unit(w, pair):
                cxt = cxp.tile([128, 512], F32, tag="cx", name=f"pv{w}_{pair}")
                first = None
                for st2 in range(2):
                    st = 4 * w + 2 * pair + st2
                    idx = 0
                    for res, xt in COMP_V:
                        wt = tw[("r" if res else "") + "wv"]
                        for ks in range(NKS):
                            m = nc.tensor.matmul(
                                cxt[:, st2 * EC:(st2 + 1) * EC],
                                xt[:, ks, :, st * 128:(st + 1) * 128],
                                wt[:, ks, :, :],
                                start=(first is None), stop=False,
                                perf_mode=DR, skip_group_check=True)
                            if first is None:
                                first = m
                            elif idx == 0:
                                order(m, first)
                            idx += 1
                for st2 in range(2):
                    st = 4 * w + 2 * pair + st2
                    nc.vector.scalar_tensor_tensor(
                        tv[:, st, :, 0:64],
                        apx(cxt, st2 * EC, [[64, 4], [1, 64]]),
                        1.0 / WSK,
                        tbv[:].rearrange("p (h d) -> p h d", h=4),
                        op0=MUL, op1=ADD)

            def proj_wave(w):
                for slot in range(2):
                    qk_unit(w, slot, "wq")
                    qk_unit(w, slot, "wk")
                for p in range(2):
                    v_unit(w, p)

            all_tes = {}

            def sct_block(qc, hp, sct, kt, c0, c1, mask0):
                # score matmuls for chunk-columns [c0, c1); if mask0, the
                # leading 128 cols are the diagonal block and get the
                # additive causal mask (0 / -1e30) folded in via I @ msk,
                # so exp emits exact zeros and no post-exp mask op exists
                for h2 in range(2):
                    hl = (2 * hp + h2) * 32
                    if mask0:
                        nc.tensor.matmul(
                            sct[:, h2, c0:c0 + 128],
                            tkf[hl:hl + 32, :, kt * 128:(kt + 1) * 128],
                            tqf[hl:hl + 32, :,
                                qc * QC + c0:qc * QC + c0 + 128],
                            start=True, stop=False, perf_mode=DR,
                            tile_position=(hl, 0))
                        nc.tensor.matmul(
                            sct[:, h2, c0:c0 + 128],
                            timat[:, :], tmsk[:, :],
                            start=False, stop=True)
                        if c0 + 128 < c1:
                            nc.tensor.matmul(
                                sct[:, h2, c0 + 128:c1],
                                tkf[hl:hl + 32, :, kt * 128:(kt + 1) * 128],
                                tqf[hl:hl + 32, :,
                                    qc * QC + c0 + 128:qc * QC + c1],
                                start=True, stop=True, perf_mode=DR,
                                tile_position=(hl, 0))
                    else:
                        nc.tensor.matmul(
                            sct[:, h2, c0:c1],
                            tkf[hl:hl + 32, :, kt * 128:(kt + 1) * 128],
                            tqf[hl:hl + 32, :,
                                qc * QC + c0:qc * QC + c1],
                            start=True, stop=True, perf_mode=DR,
                            tile_position=(hl, 0))

            def exp_block(sct, tet, c0, c1):
                nc.scalar.activation(
                    tet[:, :, c0:c1], sct[:, :, c0:c1], EXP)

            def score_unit(qc, hp, kt):
                dg = kt - 4 * qc
                coff = 128 * dg if dg > 0 else 0
                sct = scp.tile([128, 2, QC], F32, tag="sc",
                               name=f"s{qc}_{hp}_{kt}")
                tet = tep.tile([128, 2, QC], BF16, tag="te",
                               name=f"te{qc}_{hp}_{kt}")
                sct_block(qc, hp, sct, kt, coff, QC, dg >= 0)
                exp_block(sct, tet, coff, QC)
                all_tes[qc][hp][kt] = tet

            def scores_phase(qc):
                n_kt = 4 * (qc + 1)
                all_tes[qc] = [[None] * n_kt, [None] * n_kt]
                # kt outer / hp inner: ctx for sti needs (kt<=st, BOTH hp),
                # so interleaving hp lets each sti's ctx/out-proj drain
                # during the later exps instead of after the last one
                for kt in range(n_kt):
                    for hp in range(2):
                        score_unit(qc, hp, kt)

            def ctx_phase_sti(qc, sti):
                tes = all_tes[qc]
                st = 4 * qc + sti
                cxt = cxp.tile([128, 512], F32, tag="cx",
                               name=f"cx{qc}_{sti}")
                first = None
                for hp in range(2):
                    for h2 in range(2):
                        h = 2 * hp + h2
                        for kt in range(st + 1):
                            m = nc.tensor.matmul(
                                cxt[:, h * 128:h * 128 + 65],
                                tes[hp][kt][:, h2,
                                            sti * 128:(sti + 1) * 128],
                                tv[:, kt, h, :],
                                start=(first is None), stop=False,
                                skip_group_check=True)
                            if first is None:
                                first = m
                            elif kt == 0:
                                order(m, first)
                rc = rcp.tile([128, 4], F32, tag="rc",
                              name=f"rc{qc}_{sti}")
                nc.vector.reciprocal(rc[:], apx(cxt, 64, [[128, 4], [1, 1]]))
                tcn = tcnp.tile([128, 4, 64], BF16, tag="cn",
                                name=f"cn{qc}_{sti}")
                nc.vector.tensor_mul(
                    tcn[:], apx(cxt, 0, [[128, 4], [1, 64]]),
                    colb2(rc[:], 64))
                for hp in range(2):
                    nc.sync.dma_start_transpose(
                        tct[:, hp, st * 128:(st + 1) * 128],
                        tcn[:, 2 * hp:2 * hp + 2, :])

            def ctx_phase(qc):
                for sti in range(4):
                    ctx_phase_sti(qc, sti)

            def out_proj_sti(qc, sti, tail=False):
                    st = 4 * qc + sti
                    nns = (1, 0) if tail and cfg.get("tail_nn_swap") else (0, 1)
                    for nn in nns:
                        if tail and nn == 1:
                            # scores psum ring is idle in the tail; borrow it
                            # so the last out-projs don't WAR-serialize on
                            # the 2-slot prp ring
                            sc_s = scp.tile([128, 2, QC], F32, tag="sc",
                                            name=f"pys{qc}_{sti}")
                            pyt = sc_s[:, 0, :]
                        else:
                            pyt = prp.tile([128, QC], F32, tag="pr",
                                           name=f"py{qc}_{sti}_{nn}")[:]
                        for hp in range(2):
                            nc.tensor.matmul(
                                pyt,
                                tct[:, hp, st * 128:(st + 1) * 128],
                                two[:, hp, nn * QC:(nn + 1) * QC],
                                start=(hp == 0), stop=(hp == 1))
                        ty = typ.tile([128, QC], BF16, tag="y",
                                      name=f"y{qc}_{sti}_{nn}")
                        # in the tail (after the last exp) ACT is idle, so
                        # split the psum->sbuf copies across ACT and DVE
                        if tail and nn == 1:
                            nc.scalar.copy(ty[:], pyt)
                        else:
                            nc.vector.tensor_copy(ty[:], pyt)
                        yeng = (nc.scalar if tail and nn == 1
                                and cfg.get("tail_y_act", True) else nc.sync)
                        yeng.dma_start(
                            y[st * 128:(st + 1) * 128,
                              nn * QC:(nn + 1) * QC], ty[:])

            def out_proj(qc):
                for sti in range(4):
                    out_proj_sti(qc, sti)

            # Emission = scheduler priority. The exp stream is the global
            # pacer: scores/exp chains first (chunk order) with the next
            # wave's q/k projection between them, then all deferrable work.
            CORDER = cfg.get("corder", [0, 1, 3, 2])
            # waves needed before a chunk's scores: all w <= qc
            emitted_qk = set()

            def need_qk(qc):
                for w_ in range(qc + 1):
                    if w_ not in emitted_qk:
                        emitted_qk.add(w_)
                        for slot in range(2):
                            qk_unit(w_, slot, "wq")
                            qk_unit(w_, slot, "wk")

            def qk_q(slot, wn, j, prt_full):
                # wave-0 quarter projection: 128-col psum group inside a
                # shared per-(wn,slot) tile, so quarters pipeline through
                # the prp ring region-wise instead of serializing on it
                fdst, ws, bias = ((tqf, WSQ, tbq) if wn == "wq"
                                  else (tkf, WSK, tbk))
                sl = slice(j * 128, (j + 1) * 128)
                prt = prt_full[:, j * 128:(j + 1) * 128]
                idx = 0
                for res, xt in COMP_QK:
                    wt = tw[("r" if res else "") + wn]
                    for ks in range(NKS):
                        nc.tensor.matmul(
                            prt,
                            wt[:, ks, :, slot * 128:(slot + 1) * 128],
                            xt[:, ks, :, sl],
                            start=(idx == 0),
                            stop=(idx == len(COMP_QK) * NKS - 1),
                            perf_mode=DR)
                        idx += 1
                nc.vector.scalar_tensor_tensor(
                    fdst[:, slot, sl], prt, 1.0 / ws,
                    colb(bias[:, slot:slot + 1], 128), op0=MUL, op1=ADD)

            def chunk0_start():
                # wave-0 projections in 128-col quarters, interleaved with
                # chunk-0 kt0's quarter-exps: the exp stream starts as soon
                # as quarter 0 of q/k is projected instead of waiting for
                # the full 512-wide wave
                emitted_qk.add(0)
                all_tes[0] = [[None] * 4, [None] * 4]
                prts = {}
                for wn in ("wq", "wk"):
                    for slot in range(2):
                        prts[wn, slot] = prp.tile(
                            [128, QC], F32, tag="pr", name=f"p0{wn}{slot}")
                scts = {}
                tets = {}
                for slot in range(2):
                    qk_q(slot, "wq", 0, prts["wq", slot])
                for slot in range(2):
                    qk_q(slot, "wk", 0, prts["wk", slot])
                for hp in range(2):
                    scts[hp] = scp.tile([128, 2, QC], F32, tag="sc",
                                        name=f"s0_{hp}_0")
                    tets[hp] = tep.tile([128, 2, QC], BF16, tag="te",
                                        name=f"te0_{hp}_0")
                    all_tes[0][hp][0] = tets[hp]
                    sct_block(0, hp, scts[hp], 0, 0, 128, True)
                    exp_block(scts[hp], tets[hp], 0, 128)
                for j in range(1, 4):
                    for slot in range(2):
                        qk_q(slot, "wq", j, prts["wq", slot])
                    for hp in range(2):
                        sct_block(0, hp, scts[hp], 0,
                                  j * 128, (j + 1) * 128, False)
                        exp_block(scts[hp], tets[hp], j * 128, (j + 1) * 128)
                for kt in range(1, 4):
                    for slot in range(2):
                        qk_q(slot, "wk", kt, prts["wk", slot])
                    for hp in range(2):
                        score_unit(0, hp, kt)

            assert CORDER[0] == 0
            chunk0_start()
            for i, qc in enumerate(CORDER):
                if i > 0:
                    scores_phase(qc)
                if i + 1 < NQC:
                    need_qk(CORDER[i + 1])
            emitted_v = set()

            def need_v(qc):
                for w_ in range(qc + 1):
                    if w_ not in emitted_v:
                        emitted_v.add(w_)
                        for p in range(2):
                            v_unit(w_, p)

            need_v(CORDER[0])
            for i, qc in enumerate(CORDER):
                if i + 1 < NQC:
                    need_v(CORDER[i + 1])
                if i == NQC - 1:
                    # last chunk: software-pipeline out-proj one sti behind
                    # ctx, so PE never stalls in-order on a transpose-DMA
                    # sem while later A@V work is ready behind it
                    for sti in range(4):
                        ctx_phase_sti(qc, sti)
                        if sti > 0:
                            out_proj_sti(qc, sti - 1, tail=True)
                    out_proj_sti(qc, 3, tail=True)
                else:
                    ctx_phase(qc)
                    out_proj(qc)

    nc.compile()
    return nc


def _get_nc():
    if "nc" not in _CACHE:
        _CACHE["nc"] = _build_nc()
    return _CACHE["nc"]


def make_mask():
    # additive causal mask for the diagonal 128-block: 0 where attended,
    # -1e30 where masked (exp underflows to exactly 0)
    kl = np.arange(128)[:, None]
    ql = np.arange(128)[None, :]
    return np.where(ql >= kl, 0.0, -1e30).astype(BFNP)


def _fold(t):
    # [E, N] -> [128, NKS, 2, N] with e = ks*256 + sl*128 + p
    n = t.shape[1]
    return np.ascontiguousarray(
        t.reshape(NKS, 2, 128, n).transpose(2, 0, 1, 3))


def _q8(t):
    t8 = t.astype(E4NP)
    return t8, (t - t8.astype(np.float32)).astype(E4NP)


# slot-major permutation of a head-group's 256 feature columns: the
# projection matmul then emits q/k directly in the folded layout
# (partition = head_local*32 + d_low, slot = d_high).
_PERM = np.empty(EC, np.int64)
for _slot in range(2):
    for _h in range(4):
        for _dl in range(32):
            _PERM[_slot * 128 + _h * 32 + _dl] = _h * 64 + _slot * 32 + _dl


def shard_inputs(x, Wq, bq, Wk, bk, Wv, bv, Wo, bo):
    x = np.asarray(x, dtype=np.float32)
    scale = np.float32(1.0 / np.sqrt(D))
    mask = make_mask()
    ones = np.ones((1, 64), BFNP)
    in_maps = []
    xf = []
    for b in range(B):
        x8, rx8 = _q8(_fold(np.ascontiguousarray(x[b].T)))
        xf.append((x8, rx8))
    for c in range(NCORES):
        b, g = divmod(c, 4)
        cs = slice(g * EC, (g + 1) * EC)
        wq, rwq = _q8(_fold(np.asarray(Wq[:, cs])[:, _PERM] * (scale * WSQ)))
        wk, rwk = _q8(_fold(np.asarray(Wk[:, cs])[:, _PERM] * WSK))
        wv, rwv = _q8(_fold(np.asarray(Wv[:, cs]) * WSK))
        in_maps.append({
            "x8": xf[b][0], "rx8": xf[b][1],
            "wq8": wq, "rwq8": rwq,
            "wk8": wk, "rwk8": rwk,
            "wv8": wv, "rwv8": rwv,
            "wo": np.ascontiguousarray(
                np.asarray(Wo[cs, :]).reshape(2, 128, E).transpose(1, 0, 2)
            ).astype(BFNP),
            "bq": (np.asarray(bq[cs])[_PERM] * scale).reshape(2, 128, 1).astype(np.float32),
            "bk": np.asarray(bk[cs])[_PERM].reshape(2, 128, 1).astype(np.float32),
            "bv": np.asarray(bv[cs]).reshape(1, EC).astype(np.float32),
            "msk": mask,
            "imat": np.eye(128, dtype=BFNP),
            "ones": ones,
        })
    return in_maps


def merge_y(res):
    return np.asarray(res["y"], dtype=np.float32)


def combine_outputs(results, bo):
    y = np.zeros((B, S, E), np.float32)
    for c in range(NCORES):
        b = c // 4
        y[b] += merge_y(results[c])
    y += np.asarray(bo, dtype=np.float32)[None, None, :]
    return y


def kernel(x, Wq, bq, Wk, bk, Wv, bv, Wo, bo):
    from concourse.bass_utils import run_bass_kernel_spmd

    nc = _get_nc()
    in_maps = shard_inputs(x, Wq, bq, Wk, bk, Wv, bv, Wo, bo)
    try:
        res = run_bass_kernel_spmd(nc, in_maps, core_ids=list(range(NCORES)))
    except Exception:
        # transient device errors (e.g. a wedged core) usually clear on retry
        res = run_bass_kernel_spmd(nc, in_maps, core_ids=list(range(NCORES)))
    return combine_outputs(res.results, bo)



# revision 54
# speedup vs baseline: 1.0694x; 1.0025x over previous
"""Causal self-attention (B=2, S=2048, E=1024, H=16) on 8 TRN2 NeuronCores.

Sharding: core c = 4*b + g handles batch b and head-group g (4 heads,
256 E-columns). Each core computes q/k/v projections for its head slice,
causal attention for its 4 heads, and a partial output projection
y_c = ctx_g @ Wo[rows_g]. Host sums the 4 partials per batch and adds bo.

Engine plan (CoreSim cost model driven; ~93us/core vs 172.6us naive):
  PE   - QKV projections in fp8e4 DoubleRow (K=256/instr, 0.5 cyc/row) with
         3-term error compensation (x8@W8 + x8@rW8 + rx8@W8) accumulated in
         one PSUM group; q/k projections write the scores' folded fp8
         layout (partition = head_local*32 + d_low, slot = d_high) DIRECTLY
         via slot-major host-permuted weight columns - no fold DMAs;
         scores in fp8 DoubleRow (half-rate); the diagonal 128-blocks get
         an additive 0/-1e30 causal mask accumulated into PSUM via a tiny
         I @ msk matmul, so exp emits exact zeros and there is no post-exp
         mask op on any critical chain; A@V in natural orientation
         (out = [128 q-parts, 65] bf16, the denominator rides as V's ones
         column; all 4 heads of a sti share one PSUM bank via per-byte
         zero-region semantics + a nosync order edge); out-proj in bf16
         from a transposed ctxT.
  ACT  - exp (the pacer: ~8.4M exps/core ~ 75us busy incl. 185ns/instr
         PSUM/SBUF access overhead) + startup x-DMA dispatch + the tail's
         nn=1 psum->sbuf copies and y DMAs (ACT idles after its last exp).
  DVE  - q/k bias+descale (psum->fp8), v bias+descale, denominators'
         reciprocal + ctx normalize, y psum->sbuf copies.
  Pool - small-constant SWDGE DMAs + x waves 2-3.
  SP   - weight DMAs, wave-0/1 x, XBAR dma_start_transpose (ctxT), y.

Startup: wave-0 q/k projected in 128-col quarter groups packed into one
prp tile per (tensor,slot) (disjoint-region psum groups pipeline through
the ring), x/rx wave-0 DMAed per-quarter across SP+ACT queues, and chunk-0
kt0's exps issued per-quarter - first exp at ~5.5us instead of ~14us.

Emission = Tile scheduler priority: [all scores/exp chains, chunk order
0,1,3,2, hp interleaved inside each kt so ctx/out-proj drain during the
stream; next wave's q/k proj between chunks] then [v proj + ctx + out-proj
as hole-fillers]. The last chunk software-pipelines out-proj one sti
behind ctx; its copies split ACT/DVE and its nn=1 y DMAs ride ACT.

PSUM banks (8): scores ring [128,2,512]x2 = 4 (tile per (kt,hp)),
ctx/v ring [128,512]x2 = 2, qk-proj/out-proj ring [128,512]x2 = 2 (the
tail's nn=1 out-proj borrows the then-idle scores ring).
"""

import os

import numpy as np
import ml_dtypes

os.environ.setdefault("NEURON_RT_RESET_CORES", "1")

B, S, E, H, D = 2, 2048, 1024, 16, 64
NCORES = 8
EC = 256          # E-columns per core (4 heads x 64)
QC = 512          # q-chunk width
NQC = S // QC     # 4
NKT = S // 128    # 16 k-tiles
NKS = 4           # fp8 DoubleRow K-steps (4 x 256 = 1024)

WSQ = 256.0       # fp8 range scale for Wq*(1/8)
WSK = 64.0        # fp8 range scale for Wk / Wv
E4NP = ml_dtypes.float8_e4m3
BFNP = ml_dtypes.bfloat16

_CACHE = {}


def _build_nc(cfg=None):
    cfg = cfg or {}
    import concourse.mybir as mybir
    import concourse.tile as tile
    import concourse.bass as bass
    from concourse import bacc

    F32 = mybir.dt.float32
    BF16 = mybir.dt.bfloat16
    F8 = mybir.dt.float8e4
    EXP = mybir.ActivationFunctionType.Exp
    DR = mybir.MatmulPerfMode.DoubleRow
    MUL = mybir.AluOpType.mult
    ADD = mybir.AluOpType.add

    TE_BUFS = cfg.get("te", 44)
    nc = bacc.Bacc("TRN2", target_bir_lowering=False, debug=False)

    x8 = nc.dram_tensor("x8", [128, NKS, 2, S], F8, kind="ExternalInput")
    rx8 = nc.dram_tensor("rx8", [128, NKS, 2, S], F8, kind="ExternalInput")
    wq8 = nc.dram_tensor("wq8", [128, NKS, 2, EC], F8, kind="ExternalInput")
    wk8 = nc.dram_tensor("wk8", [128, NKS, 2, EC], F8, kind="ExternalInput")
    wv8 = nc.dram_tensor("wv8", [128, NKS, 2, EC], F8, kind="ExternalInput")
    rwq8 = nc.dram_tensor("rwq8", [128, NKS, 2, EC], F8, kind="ExternalInput")
    rwk8 = nc.dram_tensor("rwk8", [128, NKS, 2, EC], F8, kind="ExternalInput")
    rwv8 = nc.dram_tensor("rwv8", [128, NKS, 2, EC], F8, kind="ExternalInput")
    wo = nc.dram_tensor("wo", [128, 2, E], BF16, kind="ExternalInput")
    bq = nc.dram_tensor("bq", [2, 128, 1], F32, kind="ExternalInput")
    bk = nc.dram_tensor("bk", [2, 128, 1], F32, kind="ExternalInput")
    bv = nc.dram_tensor("bv", [1, EC], F32, kind="ExternalInput")
    msk = nc.dram_tensor("msk", [128, 128], BF16, kind="ExternalInput")
    imat = nc.dram_tensor("imat", [128, 128], BF16, kind="ExternalInput")
    ones = nc.dram_tensor("ones", [1, 64], BF16, kind="ExternalInput")

    y = nc.dram_tensor("y", [S, E], BF16, kind="ExternalOutput")

    with tile.TileContext(nc) as tc:
        with (
            tc.tile_pool(name="weights", bufs=1) as wpool,
            tc.tile_pool(name="xp", bufs=1) as xp,
            tc.tile_pool(name="qkv", bufs=1) as qkv,
            tc.tile_pool(name="tep", bufs=TE_BUFS) as tep,
            tc.tile_pool(name="tcnp", bufs=8) as tcnp,
            tc.tile_pool(name="rcp", bufs=4) as rcp,
            tc.tile_pool(name="ctp", bufs=1) as ctp,
            tc.tile_pool(name="typ", bufs=10) as typ,
            tc.tile_pool(name="smalls", bufs=1) as smalls,
            tc.tile_pool(name="scp", bufs=2, space="PSUM") as scp,
            tc.tile_pool(name="cxp", bufs=2, space="PSUM") as cxp,
            tc.tile_pool(name="prp", bufs=2, space="PSUM") as prp,
        ):
            # ---- small constants (Pool SWDGE queue) ----
            tbq = smalls.tile([128, 2], F32, tag="bq")
            tbk = smalls.tile([128, 2], F32, tag="bk")
            tbv = smalls.tile([128, EC], F32, tag="bv")
            tmsk = smalls.tile([128, 128], BF16, tag="msk")
            timat = smalls.tile([128, 128], BF16, tag="imat")

            for r in range(2):
                nc.gpsimd.dma_start(tbq[:, r:r + 1], bq[r])
                nc.gpsimd.dma_start(tbk[:, r:r + 1], bk[r])
            bvap = bv[0, :]
            bv_b = bass.AP(tensor=bvap.tensor, offset=bvap.offset,
                           ap=[[0, 128]] + list(bvap.ap))
            nc.gpsimd.dma_start(tbv[:], bv_b)
            nc.gpsimd.dma_start(tmsk[:], msk[:])
            nc.gpsimd.dma_start(timat[:], imat[:])

            # ---- x fp8 (+ residual), loaded per 512-wide wave ----
            tx = xp.tile([128, NKS, 2, S], F8, tag="x8")
            trx = xp.tile([128, NKS, 2, S], F8, tag="rx8")

            def ldx(w):
                sl = slice(w * QC, (w + 1) * QC)
                nc.gpsimd.dma_start(tx[:, :, :, sl], x8[:, :, :, sl])
                nc.gpsimd.dma_start(trx[:, :, :, sl], rx8[:, :, :, sl])

            # wave-0 x + q/k weights first so the q/k chain starts ASAP.
            # Weights dispatch on SP, wave-0 x/rx on DVE (both idle at start)
            # so neither queue serializes the first-exp critical path.
            sl0 = slice(0, QC)
            tw = {}

            def ldw(nm, dram, eng=None):
                t = wpool.tile([128, NKS, 2, EC], F8, tag=nm, name=nm)
                (eng or nc.sync).dma_start(t[:], dram[:])
                tw[nm] = t

            # startup: wave-0 x/rx arrive in 128-col quarters, spread over
            # the SP and ACT HWDGE queues so quarter 0 + the q/k weights
            # land as early as possible for the first quarter-projection
            def ldxq(eng, xt, dram, j):
                eng.dma_start(xt[:, :, :, j * 128:(j + 1) * 128],
                              dram[:, :, :, j * 128:(j + 1) * 128])

            ldxq(nc.sync, tx, x8, 0)
            ldw("wq", wq8)
            ldxq(nc.scalar, trx, rx8, 0)
            ldw("wk", wk8)
            ldw("rwk", rwk8, eng=nc.scalar)
            ldw("rwq", rwq8)
            for j in range(1, 4):
                ldxq(nc.scalar, trx, rx8, j)
            for j in range(1, 4):
                ldxq(nc.sync, tx, x8, j)
            # wave-1 x on SP so its q/k projection is ready the moment
            # chunk-0's exps drain (Pool's SWDGE queue is too slow for it)
            sl1 = slice(QC, 2 * QC)
            nc.sync.dma_start(tx[:, :, :, sl1], x8[:, :, :, sl1])
            nc.sync.dma_start(trx[:, :, :, sl1], rx8[:, :, :, sl1])
            ldw("wv", wv8)
            ldw("rwv", rwv8)
            two = wpool.tile([128, 2, E], BF16, tag="wo")
            nc.sync.dma_start(two[:], wo[:])
            for w_ in range(2, NQC):
                ldx(w_)

            # ---- persistent activations ----
            # folded fp8 layout: partition = head_local*32 + d_low,
            # slot = d_high half; written DIRECTLY by the projection
            # (weight columns are slot-major-permuted on the host), so no
            # partition-shift fold DMAs are needed.
            tqf = qkv.tile([128, 2, S], F8, tag="qf", name="qf")
            tkf = qkv.tile([128, 2, S], F8, tag="kf", name="kf")
            # v1: [128, kt, head, 65]; col 64 of each head block = 1.0
            tv = qkv.tile([128, NKT, 4, 65], BF16, tag="v")
            onesap = ones[0, 0:1]
            ones_v = bass.AP(tensor=onesap.tensor, offset=onesap.offset,
                             ap=[[0, 128], [0, NKT * 4], [0, 1]])
            nc.gpsimd.dma_start(tv[:, :, :, 64:65], ones_v)

            tct = ctp.tile([128, 2, S], BF16, tag="ct")  # folded ctxT

            def colb(ap1, n):
                # [128, 1] AP -> [128, n] stride-0 broadcast
                return bass.AP(tensor=ap1.tensor, offset=ap1.offset,
                               ap=[list(ap1.ap[0]), [0, n]])

            def colb2(ap2, n):
                # [128, k] AP -> [128, k, n] stride-0 broadcast
                return bass.AP(tensor=ap2.tensor, offset=ap2.offset,
                               ap=[list(ap2.ap[0]), list(ap2.ap[1]), [0, n]])

            def apx(t, off, dims):
                # partition dim of tile t + custom free dims at f32-col offset
                a = t[:]
                return bass.AP(tensor=a.tensor, offset=a.offset + off,
                               ap=[list(a.ap[0])] + [list(d) for d in dims])

            def order(later, first):
                bass._add_dep_helper(later.ins, first.ins, sync=False,
                                     reason="psum zero-region order")

            COMP = ((None, tx), ("r", tx), (None, trx))  # (w-residual?, x-tensor)
            COMP_QK = COMP[:2] if cfg.get("qk2") else COMP
            COMP_V = COMP[:2] if cfg.get("v2") else COMP

            def qk_unit(w, slot, wn, q4=None):
                fdst, ws, bias = ((tqf, WSQ, tbq) if wn == "wq"
                                  else (tkf, WSK, tbk))
                if q4 is None:
                    sl = slice(w * QC, (w + 1) * QC)
                    width = QC
                else:
                    sl = slice(w * QC + q4 * 128, w * QC + (q4 + 1) * 128)
                    width = 128
                prt = prp.tile([128, QC], F32, tag="pr",
                               name=f"p{wn}{w}_{slot}_{q4}")[:, 0:width]
                idx = 0
                for res, xt in COMP_QK:
                    wt = tw[("r" if res else "") + wn]
                    for ks in range(NKS):
                        nc.tensor.matmul(
                            prt,
                            wt[:, ks, :, slot * 128:(slot + 1) * 128],
                            xt[:, ks, :, sl],
                            start=(idx == 0),
                            stop=(idx == len(COMP_QK) * NKS - 1),
                            perf_mode=DR)
                        idx += 1
                nc.vector.scalar_tensor_tensor(
                    fdst[:, slot, sl], prt, 1.0 / ws,
                    colb(bias[:, slot:slot + 1], width), op0=MUL, op1=ADD)

            def v_unit(w, pair):
                cxt = cxp.tile([128, 512], F32, tag="cx", name=f"pv{w}_{pair}")
                first = None
                for st2 in range(2):
                    st = 4 * w + 2 * pair + st2
                    idx = 0
                    for res, xt in COMP_V:
                        wt = tw[("r" if res else "") + "wv"]
                        for ks in range(NKS):
                            m = nc.tensor.matmul(
                                cxt[:, st2 * EC:(st2 + 1) * EC],
                                xt[:, ks, :, st * 128:(st + 1) * 128],
                                wt[:, ks, :, :],
                                start=(first is None), stop=False,
                                perf_mode=DR, skip_group_check=True)
                            if first is None:
                                first = m
                            elif idx == 0:
                                order(m, first)
                            idx += 1
                for st2 in range(2):
                    st = 4 * w + 2 * pair + st2
                    nc.vector.scalar_tensor_tensor(
                        tv[:, st, :, 0:64],
                        apx(cxt, st2 * EC, [[64, 4], [1, 64]]),
                        1.0 / WSK,
                        tbv[:].rearrange("p (h d) -> p h d", h=4),
                        op0=MUL, op1=ADD)

            def proj_wave(w):
                for slot in range(2):
                    qk_unit(w, slot, "wq")
                    qk_unit(w, slot, "wk")
                for p in range(2):
                    v_unit(w, p)

            all_tes = {}

            def sct_block(qc, hp, sct, kt, c0, c1, mask0):
                # score matmuls for chunk-columns [c0, c1); if mask0, the
                # leading 128 cols are the diagonal block and get the
                # additive causal mask (0 / -1e30) folded in via I @ msk,
                # so exp emits exact zeros and no post-exp mask op exists
                for h2 in range(2):
                    hl = (2 * hp + h2) * 32
                    if mask0:
                        nc.tensor.matmul(
                            sct[:, h2, c0:c0 + 128],
                            tkf[hl:hl + 32, :, kt * 128:(kt + 1) * 128],
                            tqf[hl:hl + 32, :,
                                qc * QC + c0:qc * QC + c0 + 128],
                            start=True, stop=False, perf_mode=DR,
                            tile_position=(hl, 0))
                        nc.tensor.matmul(
                            sct[:, h2, c0:c0 + 128],
                            timat[:, :], tmsk[:, :],
                            start=False, stop=True)
                        if c0 + 128 < c1:
                            nc.tensor.matmul(
                                sct[:, h2, c0 + 128:c1],
                                tkf[hl:hl + 32, :, kt * 128:(kt + 1) * 128],
                                tqf[hl:hl + 32, :,
                                    qc * QC + c0 + 128:qc * QC + c1],
                                start=True, stop=True, perf_mode=DR,
                                tile_position=(hl, 0))
                    else:
                        nc.tensor.matmul(
                            sct[:, h2, c0:c1],
                            tkf[hl:hl + 32, :, kt * 128:(kt + 1) * 128],
                            tqf[hl:hl + 32, :,
                                qc * QC + c0:qc * QC + c1],
                            start=True, stop=True, perf_mode=DR,
                            tile_position=(hl, 0))

            def exp_block(sct, tet, c0, c1):
                nc.scalar.activation(
                    tet[:, :, c0:c1], sct[:, :, c0:c1], EXP)

            def score_unit(qc, hp, kt):
                dg = kt - 4 * qc
                coff = 128 * dg if dg > 0 else 0
                sct = scp.tile([128, 2, QC], F32, tag="sc",
                               name=f"s{qc}_{hp}_{kt}")
                tet = tep.tile([128, 2, QC], BF16, tag="te",
                               name=f"te{qc}_{hp}_{kt}")
                sct_block(qc, hp, sct, kt, coff, QC, dg >= 0)
                exp_block(sct, tet, coff, QC)
                all_tes[qc][hp][kt] = tet

            def scores_phase(qc):
                n_kt = 4 * (qc + 1)
                all_tes[qc] = [[None] * n_kt, [None] * n_kt]
                # kt outer / hp inner: ctx for sti needs (kt<=st, BOTH hp),
                # so interleaving hp lets each sti's ctx/out-proj drain
                # during the later exps instead of after the last one
                for kt in range(n_kt):
                    for hp in range(2):
                        score_unit(qc, hp, kt)

            def ctx_phase_sti(qc, sti):
                tes = all_tes[qc]
                st = 4 * qc + sti
                cxt = cxp.tile([128, 512], F32, tag="cx",
                               name=f"cx{qc}_{sti}")
                first = None
                for hp in range(2):
                    for h2 in range(2):
                        h = 2 * hp + h2
                        for kt in range(st + 1):
                            m = nc.tensor.matmul(
                                cxt[:, h * 128:h * 128 + 65],
                                tes[hp][kt][:, h2,
                                            sti * 128:(sti + 1) * 128],
                                tv[:, kt, h, :],
                                start=(first is None), stop=False,
                                skip_group_check=True)
                            if first is None:
                                first = m
                            elif kt == 0:
                                order(m, first)
                rc = rcp.tile([128, 4], F32, tag="rc",
                              name=f"rc{qc}_{sti}")
                nc.vector.reciprocal(rc[:], apx(cxt, 64, [[128, 4], [1, 1]]))
                tcn = tcnp.tile([128, 4, 64], BF16, tag="cn",
                                name=f"cn{qc}_{sti}")
                nc.vector.tensor_mul(
                    tcn[:], apx(cxt, 0, [[128, 4], [1, 64]]),
                    colb2(rc[:], 64))
                for hp in range(2):
                    nc.sync.dma_start_transpose(
                        tct[:, hp, st * 128:(st + 1) * 128],
                        tcn[:, 2 * hp:2 * hp + 2, :])

            def ctx_phase(qc):
                for sti in range(4):
                    ctx_phase_sti(qc, sti)

            def out_proj_sti(qc, sti, tail=False):
                    st = 4 * qc + sti
                    nns = ((1, 0) if tail and cfg.get("tail_nn_swap", True)
                           else (0, 1))
                    for nn in nns:
                        if tail and nn == 1:
                            # scores psum ring is idle in the tail; borrow it
                            # so the last out-projs don't WAR-serialize on
                            # the 2-slot prp ring
                            sc_s = scp.tile([128, 2, QC], F32, tag="sc",
                                            name=f"pys{qc}_{sti}")
                            pyt = sc_s[:, 0, :]
                        else:
                            pyt = prp.tile([128, QC], F32, tag="pr",
                                           name=f"py{qc}_{sti}_{nn}")[:]
                        for hp in range(2):
                            nc.tensor.matmul(
                                pyt,
                                tct[:, hp, st * 128:(st + 1) * 128],
                                two[:, hp, nn * QC:(nn + 1) * QC],
                                start=(hp == 0), stop=(hp == 1))
                        ty = typ.tile([128, QC], BF16, tag="y",
                                      name=f"y{qc}_{sti}_{nn}")
                        # in the tail (after the last exp) ACT is idle, so
                        # split the psum->sbuf copies across ACT and DVE
                        if tail and nn == 1:
                            nc.scalar.copy(ty[:], pyt)
                        else:
                            nc.vector.tensor_copy(ty[:], pyt)
                        yeng = (nc.scalar if tail and nn == 1
                                and cfg.get("tail_y_act", True) else nc.sync)
                        yeng.dma_start(
                            y[st * 128:(st + 1) * 128,
                              nn * QC:(nn + 1) * QC], ty[:])

            def out_proj(qc):
                for sti in range(4):
                    out_proj_sti(qc, sti)

            # Emission = scheduler priority. The exp stream is the global
            # pacer: scores/exp chains first (chunk order) with the next
            # wave's q/k projection between them, then all deferrable work.
            CORDER = cfg.get("corder", [0, 1, 3, 2])
            # waves needed before a chunk's scores: all w <= qc
            emitted_qk = set()

            def need_qk(qc):
                for w_ in range(qc + 1):
                    if w_ not in emitted_qk:
                        emitted_qk.add(w_)
                        for slot in range(2):
                            qk_unit(w_, slot, "wq")
                            qk_unit(w_, slot, "wk")

            def qk_q(slot, wn, j, prt_full):
                # wave-0 quarter projection: 128-col psum group inside a
                # shared per-(wn,slot) tile, so quarters pipeline through
                # the prp ring region-wise instead of serializing on it
                fdst, ws, bias = ((tqf, WSQ, tbq) if wn == "wq"
                                  else (tkf, WSK, tbk))
                sl = slice(j * 128, (j + 1) * 128)
                prt = prt_full[:, j * 128:(j + 1) * 128]
                idx = 0
                for res, xt in COMP_QK:
                    wt = tw[("r" if res else "") + wn]
                    for ks in range(NKS):
                        nc.tensor.matmul(
                            prt,
                            wt[:, ks, :, slot * 128:(slot + 1) * 128],
                            xt[:, ks, :, sl],
                            start=(idx == 0),
                            stop=(idx == len(COMP_QK) * NKS - 1),
                            perf_mode=DR)
                        idx += 1
                nc.vector.scalar_tensor_tensor(
                    fdst[:, slot, sl], prt, 1.0 / ws,
                    colb(bias[:, slot:slot + 1], 128), op0=MUL, op1=ADD)

            def chunk0_start():
                # wave-0 projections in 128-col quarters, interleaved with
                # chunk-0 kt0's quarter-exps: the exp stream starts as soon
                # as quarter 0 of q/k is projected instead of waiting for
                # the full 512-wide wave
                emitted_qk.add(0)
                all_tes[0] = [[None] * 4, [None] * 4]
                prts = {}
                for wn in ("wq", "wk"):
                    for slot in range(2):
                        prts[wn, slot] = prp.tile(
                            [128, QC], F32, tag="pr", name=f"p0{wn}{slot}")
                scts = {}
                tets = {}
                for slot in range(2):
                    qk_q(slot, "wq", 0, prts["wq", slot])
                for slot in range(2):
                    qk_q(slot, "wk", 0, prts["wk", slot])
                hps = (0,) if cfg.get("hp1_coarse") else (0, 1)
                for hp in hps:
                    scts[hp] = scp.tile([128, 2, QC], F32, tag="sc",
                                        name=f"s0_{hp}_0")
                    tets[hp] = tep.tile([128, 2, QC], BF16, tag="te",
                                        name=f"te0_{hp}_0")
                    all_tes[0][hp][0] = tets[hp]
                    sct_block(0, hp, scts[hp], 0, 0, 128, True)
                    exp_block(scts[hp], tets[hp], 0, 128)
                for j in range(1, 4):
                    for slot in range(2):
                        qk_q(slot, "wq", j, prts["wq", slot])
                    for hp in hps:
                        sct_block(0, hp, scts[hp], 0,
                                  j * 128, (j + 1) * 128, False)
                        exp_block(scts[hp], tets[hp], j * 128, (j + 1) * 128)
                if cfg.get("hp1_coarse"):
                    score_unit(0, 1, 0)
                for kt in range(1, 4):
                    for slot in range(2):
                        qk_q(slot, "wk", kt, prts["wk", slot])
                    for hp in range(2):
                        score_unit(0, hp, kt)

            assert CORDER[0] == 0
            chunk0_start()
            for i, qc in enumerate(CORDER):
                if i > 0:
                    scores_phase(qc)
                if i + 1 < NQC:
                    need_qk(CORDER[i + 1])
            emitted_v = set()

            def need_v(qc):
                for w_ in range(qc + 1):
                    if w_ not in emitted_v:
                        emitted_v.add(w_)
                        for p in range(2):
                            v_unit(w_, p)

            need_v(CORDER[0])
            for i, qc in enumerate(CORDER):
                if i + 1 < NQC:
                    need_v(CORDER[i + 1])
                if i == NQC - 1:
                    # last chunk: software-pipeline out-proj one sti behind
                    # ctx, so PE never stalls in-order on a transpose-DMA
                    # sem while later A@V work is ready behind it
                    for sti in range(4):
                        ctx_phase_sti(qc, sti)
                        if sti > 0:
                            out_proj_sti(qc, sti - 1, tail=True)
                    out_proj_sti(qc, 3, tail=True)
                else:
                    ctx_phase(qc)
                    out_proj(qc)

    nc.compile()
    return nc


def _get_nc():
    if "nc" not in _CACHE:
        _CACHE["nc"] = _build_nc()
    return _CACHE["nc"]


def make_mask():
    # additive causal mask for the diagonal 128-block: 0 where attended,
    # -1e30 where masked (exp underflows to exactly 0)
    kl = np.arange(128)[:, None]
    ql = np.arange(128)[None, :]
    return np.where(ql >= kl, 0.0, -1e30).astype(BFNP)


def _fold(t):
    # [E, N] -> [128, NKS, 2, N] with e = ks*256 + sl*128 + p
    n = t.shape[1]
    return np.ascontiguousarray(
        t.reshape(NKS, 2, 128, n).transpose(2, 0, 1, 3))


def _q8(t):
    t8 = t.astype(E4NP)
    return t8, (t - t8.astype(np.float32)).astype(E4NP)


# slot-major permutation of a head-group's 256 feature columns: the
# projection matmul then emits q/k directly in the folded layout
# (partition = head_local*32 + d_low, slot = d_high).
_PERM = np.empty(EC, np.int64)
for _slot in range(2):
    for _h in range(4):
        for _dl in range(32):
            _PERM[_slot * 128 + _h * 32 + _dl] = _h * 64 + _slot * 32 + _dl


def shard_inputs(x, Wq, bq, Wk, bk, Wv, bv, Wo, bo):
    x = np.asarray(x, dtype=np.float32)
    scale = np.float32(1.0 / np.sqrt(D))
    mask = make_mask()
    ones = np.ones((1, 64), BFNP)
    in_maps = []
    xf = []
    for b in range(B):
        x8, rx8 = _q8(_fold(np.ascontiguousarray(x[b].T)))
        xf.append((x8, rx8))
    for c in range(NCORES):
        b, g = divmod(c, 4)
        cs = slice(g * EC, (g + 1) * EC)
        wq, rwq = _q8(_fold(np.asarray(Wq[:, cs])[:, _PERM] * (scale * WSQ)))
        wk, rwk = _q8(_fold(np.asarray(Wk[:, cs])[:, _PERM] * WSK))
        wv, rwv = _q8(_fold(np.asarray(Wv[:, cs]) * WSK))
        in_maps.append({
            "x8": xf[b][0], "rx8": xf[b][1],
            "wq8": wq, "rwq8": rwq,
            "wk8": wk, "rwk8": rwk,
            "wv8": wv, "rwv8": rwv,
            "wo": np.ascontiguousarray(
                np.asarray(Wo[cs, :]).reshape(2, 128, E).transpose(1, 0, 2)
            ).astype(BFNP),
            "bq": (np.asarray(bq[cs])[_PERM] * scale).reshape(2, 128, 1).astype(np.float32),
            "bk": np.asarray(bk[cs])[_PERM].reshape(2, 128, 1).astype(np.float32),
            "bv": np.asarray(bv[cs]).reshape(1, EC).astype(np.float32),
            "msk": mask,
            "imat": np.eye(128, dtype=BFNP),
            "ones": ones,
        })
    return in_maps


def merge_y(res):
    return np.asarray(res["y"], dtype=np.float32)


def combine_outputs(results, bo):
    y = np.zeros((B, S, E), np.float32)
    for c in range(NCORES):
        b = c // 4
        y[b] += merge_y(results[c])
    y += np.asarray(bo, dtype=np.float32)[None, None, :]
    return y


def kernel(x, Wq, bq, Wk, bk, Wv, bv, Wo, bo):
    from concourse.bass_utils import run_bass_kernel_spmd

    nc = _get_nc()
    in_maps = shard_inputs(x, Wq, bq, Wk, bk, Wv, bv, Wo, bo)
    try:
        res = run_bass_kernel_spmd(nc, in_maps, core_ids=list(range(NCORES)))
    except Exception:
        # transient device errors (e.g. a wedged core) usually clear on retry
        res = run_bass_kernel_spmd(nc, in_maps, core_ids=list(range(NCORES)))
    return combine_outputs(res.results, bo)



# revision 62
# speedup vs baseline: 1.0754x; 1.0056x over previous
"""Causal self-attention (B=2, S=2048, E=1024, H=16) on 8 TRN2 NeuronCores.

Sharding: core c = 4*b + g handles batch b and head-group g (4 heads,
256 E-columns). Each core computes q/k/v projections for its head slice,
causal attention for its 4 heads, and a partial output projection
y_c = ctx_g @ Wo[rows_g]. Host sums the 4 partials per batch and adds bo.

Engine plan (CoreSim cost model driven; ~93us/core vs 172.6us naive):
  PE   - QKV projections in fp8e4 DoubleRow (K=256/instr, 0.5 cyc/row) with
         3-term error compensation (x8@W8 + x8@rW8 + rx8@W8) accumulated in
         one PSUM group; q/k projections write the scores' folded fp8
         layout (partition = head_local*32 + d_low, slot = d_high) DIRECTLY
         via slot-major host-permuted weight columns - no fold DMAs;
         scores in fp8 DoubleRow (half-rate); the diagonal 128-blocks get
         an additive 0/-1e30 causal mask accumulated into PSUM via a tiny
         I @ msk matmul, so exp emits exact zeros and there is no post-exp
         mask op on any critical chain; A@V in natural orientation
         (out = [128 q-parts, 65] bf16, the denominator rides as V's ones
         column; all 4 heads of a sti share one PSUM bank via per-byte
         zero-region semantics + a nosync order edge); out-proj in bf16
         from a transposed ctxT.
  ACT  - exp (the pacer: ~8.4M exps/core ~ 75us busy incl. 185ns/instr
         PSUM/SBUF access overhead) + startup x-DMA dispatch + the tail's
         nn=1 psum->sbuf copies and y DMAs (ACT idles after its last exp).
  DVE  - q/k bias+descale (psum->fp8), v bias+descale, denominators'
         reciprocal + ctx normalize, y psum->sbuf copies.
  Pool - small-constant SWDGE DMAs + x waves 2-3.
  SP   - weight DMAs, wave-0/1 x, XBAR dma_start_transpose (ctxT), y.

Startup: wave-0 q/k projected in 128-col quarter groups packed into one
prp tile per (tensor,slot) (disjoint-region psum groups pipeline through
the ring), x/rx wave-0 DMAed per-quarter across SP+ACT queues, and chunk-0
kt0's exps issued per-quarter - first exp at ~5.5us instead of ~14us.

Emission = Tile scheduler priority: [all scores/exp chains, chunk order
0,1,3,2, hp interleaved inside each kt so ctx/out-proj drain during the
stream; next wave's q/k proj between chunks] then [v proj + ctx + out-proj
as hole-fillers]. The last chunk software-pipelines out-proj one sti
behind ctx; its copies split ACT/DVE and its nn=1 y DMAs ride ACT.

PSUM banks (8): scores ring [128,2,512]x2 = 4 (tile per (kt,hp)),
ctx/v ring [128,512]x2 = 2, qk-proj/out-proj ring [128,512]x2 = 2 (the
tail's nn=1 out-proj borrows the then-idle scores ring).
"""

import os

import numpy as np
import ml_dtypes

os.environ.setdefault("NEURON_RT_RESET_CORES", "1")

B, S, E, H, D = 2, 2048, 1024, 16, 64
NCORES = 8
EC = 256          # E-columns per core (4 heads x 64)
QC = 512          # q-chunk width
NQC = S // QC     # 4
NKT = S // 128    # 16 k-tiles
NKS = 4           # fp8 DoubleRow K-steps (4 x 256 = 1024)

WSQ = 256.0       # fp8 range scale for Wq*(1/8)
WSK = 64.0        # fp8 range scale for Wk / Wv
E4NP = ml_dtypes.float8_e4m3
BFNP = ml_dtypes.bfloat16

_CACHE = {}


def _build_nc(cfg=None):
    cfg = cfg or {}
    import concourse.mybir as mybir
    import concourse.tile as tile
    import concourse.bass as bass
    from concourse import bacc

    F32 = mybir.dt.float32
    BF16 = mybir.dt.bfloat16
    F8 = mybir.dt.float8e4
    EXP = mybir.ActivationFunctionType.Exp
    DR = mybir.MatmulPerfMode.DoubleRow
    MUL = mybir.AluOpType.mult
    ADD = mybir.AluOpType.add

    TE_BUFS = cfg.get("te", 44)
    nc = bacc.Bacc("TRN2", target_bir_lowering=False, debug=False)

    x8 = nc.dram_tensor("x8", [128, NKS, 2, S], F8, kind="ExternalInput")
    rx8 = nc.dram_tensor("rx8", [128, NKS, 2, S], F8, kind="ExternalInput")
    wq8 = nc.dram_tensor("wq8", [128, NKS, 2, EC], F8, kind="ExternalInput")
    wk8 = nc.dram_tensor("wk8", [128, NKS, 2, EC], F8, kind="ExternalInput")
    wv8 = nc.dram_tensor("wv8", [128, NKS, 2, EC], F8, kind="ExternalInput")
    rwq8 = nc.dram_tensor("rwq8", [128, NKS, 2, EC], F8, kind="ExternalInput")
    rwk8 = nc.dram_tensor("rwk8", [128, NKS, 2, EC], F8, kind="ExternalInput")
    rwv8 = nc.dram_tensor("rwv8", [128, NKS, 2, EC], F8, kind="ExternalInput")
    wo = nc.dram_tensor("wo", [128, 2, E], BF16, kind="ExternalInput")
    bq = nc.dram_tensor("bq", [2, 128, 1], F32, kind="ExternalInput")
    bk = nc.dram_tensor("bk", [2, 128, 1], F32, kind="ExternalInput")
    bv = nc.dram_tensor("bv", [1, EC], F32, kind="ExternalInput")
    msk = nc.dram_tensor("msk", [128, 128], BF16, kind="ExternalInput")
    imat = nc.dram_tensor("imat", [128, 128], BF16, kind="ExternalInput")
    ones = nc.dram_tensor("ones", [1, 64], BF16, kind="ExternalInput")

    y = nc.dram_tensor("y", [S, E], BF16, kind="ExternalOutput")

    with tile.TileContext(nc) as tc:
        with (
            tc.tile_pool(name="weights", bufs=1) as wpool,
            tc.tile_pool(name="xp", bufs=1) as xp,
            tc.tile_pool(name="qkv", bufs=1) as qkv,
            tc.tile_pool(name="tep", bufs=TE_BUFS) as tep,
            tc.tile_pool(name="tcnp", bufs=8) as tcnp,
            tc.tile_pool(name="rcp", bufs=4) as rcp,
            tc.tile_pool(name="ctp", bufs=1) as ctp,
            tc.tile_pool(name="typ", bufs=10) as typ,
            tc.tile_pool(name="smalls", bufs=1) as smalls,
            tc.tile_pool(name="scp", bufs=2, space="PSUM") as scp,
            tc.tile_pool(name="cxp", bufs=2, space="PSUM") as cxp,
            tc.tile_pool(name="prp", bufs=2, space="PSUM") as prp,
        ):
            # ---- small constants (Pool SWDGE queue) ----
            tbq = smalls.tile([128, 2], F32, tag="bq")
            tbk = smalls.tile([128, 2], F32, tag="bk")
            tbv = smalls.tile([128, EC], F32, tag="bv")
            tmsk = smalls.tile([128, 128], BF16, tag="msk")
            timat = smalls.tile([128, 128], BF16, tag="imat")

            for r in range(2):
                nc.gpsimd.dma_start(tbq[:, r:r + 1], bq[r])
                nc.gpsimd.dma_start(tbk[:, r:r + 1], bk[r])
            bvap = bv[0, :]
            bv_b = bass.AP(tensor=bvap.tensor, offset=bvap.offset,
                           ap=[[0, 128]] + list(bvap.ap))
            nc.gpsimd.dma_start(tbv[:], bv_b)
            nc.gpsimd.dma_start(tmsk[:], msk[:])
            nc.gpsimd.dma_start(timat[:], imat[:])

            # ---- x fp8 (+ residual), loaded per 512-wide wave ----
            tx = xp.tile([128, NKS, 2, S], F8, tag="x8")
            trx = xp.tile([128, NKS, 2, S], F8, tag="rx8")

            def ldx(w):
                sl = slice(w * QC, (w + 1) * QC)
                nc.gpsimd.dma_start(tx[:, :, :, sl], x8[:, :, :, sl])
                nc.gpsimd.dma_start(trx[:, :, :, sl], rx8[:, :, :, sl])

            # wave-0 x + q/k weights first so the q/k chain starts ASAP.
            # Weights dispatch on SP, wave-0 x/rx on DVE (both idle at start)
            # so neither queue serializes the first-exp critical path.
            sl0 = slice(0, QC)
            tw = {}

            def ldw(nm, dram, eng=None):
                t = wpool.tile([128, NKS, 2, EC], F8, tag=nm, name=nm)
                (eng or nc.sync).dma_start(t[:], dram[:])
                tw[nm] = t

            # startup: wave-0 x/rx arrive in 128-col quarters, spread over
            # the SP and ACT HWDGE queues so quarter 0 + the q/k weights
            # land as early as possible for the first quarter-projection
            def ldxq(eng, xt, dram, j):
                eng.dma_start(xt[:, :, :, j * 128:(j + 1) * 128],
                              dram[:, :, :, j * 128:(j + 1) * 128])

            ldxq(nc.sync, tx, x8, 0)
            ldw("wq", wq8)
            ldxq(nc.scalar, trx, rx8, 0)
            ldw("wk", wk8)
            ldw("rwk", rwk8, eng=nc.scalar)
            ldw("rwq", rwq8)
            for j in range(1, 4):
                ldxq(nc.scalar, trx, rx8, j)
            for j in range(1, 4):
                ldxq(nc.sync, tx, x8, j)
            # wave-1 x on SP so its q/k projection is ready the moment
            # chunk-0's exps drain (Pool's SWDGE queue is too slow for it)
            sl1 = slice(QC, 2 * QC)
            nc.sync.dma_start(tx[:, :, :, sl1], x8[:, :, :, sl1])
            nc.sync.dma_start(trx[:, :, :, sl1], rx8[:, :, :, sl1])
            ldw("wv", wv8)
            ldw("rwv", rwv8)
            two = wpool.tile([128, 2, E], BF16, tag="wo")
            nc.sync.dma_start(two[:], wo[:])
            for w_ in range(2, NQC):
                ldx(w_)

            # ---- PE clock warmup ----
            # the cost model ramps PE to full clock only after 3us of
            # continuous busy; without this, the whole startup projection
            # phase runs at half clock. Chain a few dummy matmuls on a
            # zeroed scratch tile from t~0.7us so the real matmuls (~3.5us)
            # run at full speed.
            NWARM = cfg.get("warm", 0)
            if NWARM:
                tscr = smalls.tile([128, QC], BF16, tag="scr")
                nc.vector.memzero(tscr[:])
                pwt = prp.tile([128, QC], F32, tag="pr", name="pwarm")
                for _ in range(NWARM):
                    nc.tensor.matmul(pwt[:], tscr[:, 0:128], tscr[:],
                                     start=True, stop=True)

            # ---- persistent activations ----
            # folded fp8 layout: partition = head_local*32 + d_low,
            # slot = d_high half; written DIRECTLY by the projection
            # (weight columns are slot-major-permuted on the host), so no
            # partition-shift fold DMAs are needed.
            tqf = qkv.tile([128, 2, S], F8, tag="qf", name="qf")
            tkf = qkv.tile([128, 2, S], F8, tag="kf", name="kf")
            # v1: [128, kt, head, 65]; col 64 of each head block = 1.0
            tv = qkv.tile([128, NKT, 4, 65], BF16, tag="v")
            onesap = ones[0, 0:1]
            ones_v = bass.AP(tensor=onesap.tensor, offset=onesap.offset,
                             ap=[[0, 128], [0, NKT * 4], [0, 1]])
            nc.gpsimd.dma_start(tv[:, :, :, 64:65], ones_v)

            tct = ctp.tile([128, 2, S], BF16, tag="ct")  # folded ctxT

            def colb(ap1, n):
                # [128, 1] AP -> [128, n] stride-0 broadcast
                return bass.AP(tensor=ap1.tensor, offset=ap1.offset,
                               ap=[list(ap1.ap[0]), [0, n]])

            def colb2(ap2, n):
                # [128, k] AP -> [128, k, n] stride-0 broadcast
                return bass.AP(tensor=ap2.tensor, offset=ap2.offset,
                               ap=[list(ap2.ap[0]), list(ap2.ap[1]), [0, n]])

            def apx(t, off, dims):
                # partition dim of tile t + custom free dims at f32-col offset
                a = t[:]
                return bass.AP(tensor=a.tensor, offset=a.offset + off,
                               ap=[list(a.ap[0])] + [list(d) for d in dims])

            def order(later, first):
                bass._add_dep_helper(later.ins, first.ins, sync=False,
                                     reason="psum zero-region order")

            COMP = ((None, tx), ("r", tx), (None, trx))  # (w-residual?, x-tensor)
            COMP_QK = COMP[:2] if cfg.get("qk2") else COMP
            COMP_V = COMP[:2] if cfg.get("v2") else COMP

            def qk_unit(w, slot, wn, q4=None):
                fdst, ws, bias = ((tqf, WSQ, tbq) if wn == "wq"
                                  else (tkf, WSK, tbk))
                if q4 is None:
                    sl = slice(w * QC, (w + 1) * QC)
                    width = QC
                else:
                    sl = slice(w * QC + q4 * 128, w * QC + (q4 + 1) * 128)
                    width = 128
                prt = prp.tile([128, QC], F32, tag="pr",
                               name=f"p{wn}{w}_{slot}_{q4}")[:, 0:width]
                idx = 0
                for res, xt in COMP_QK:
                    wt = tw[("r" if res else "") + wn]
                    for ks in range(NKS):
                        nc.tensor.matmul(
                            prt,
                            wt[:, ks, :, slot * 128:(slot + 1) * 128],
                            xt[:, ks, :, sl],
                            start=(idx == 0),
                            stop=(idx == len(COMP_QK) * NKS - 1),
                            perf_mode=DR)
                        idx += 1
                nc.vector.scalar_tensor_tensor(
                    fdst[:, slot, sl], prt, 1.0 / ws,
                    colb(bias[:, slot:slot + 1], width), op0=MUL, op1=ADD)

            def v_unit(w, pair):
                cxt = cxp.tile([128, 512], F32, tag="cx", name=f"pv{w}_{pair}")
                first = None
                for st2 in range(2):
                    st = 4 * w + 2 * pair + st2
                    idx = 0
                    for res, xt in COMP_V:
                        wt = tw[("r" if res else "") + "wv"]
                        for ks in range(NKS):
                            m = nc.tensor.matmul(
                                cxt[:, st2 * EC:(st2 + 1) * EC],
                                xt[:, ks, :, st * 128:(st + 1) * 128],
                                wt[:, ks, :, :],
                                start=(first is None), stop=False,
                                perf_mode=DR, skip_group_check=True)
                            if first is None:
                                first = m
                            elif idx == 0:
                                order(m, first)
                            idx += 1
                for st2 in range(2):
                    st = 4 * w + 2 * pair + st2
                    nc.vector.scalar_tensor_tensor(
                        tv[:, st, :, 0:64],
                        apx(cxt, st2 * EC, [[64, 4], [1, 64]]),
                        1.0 / WSK,
                        tbv[:].rearrange("p (h d) -> p h d", h=4),
                        op0=MUL, op1=ADD)

            def proj_wave(w):
                for slot in range(2):
                    qk_unit(w, slot, "wq")
                    qk_unit(w, slot, "wk")
                for p in range(2):
                    v_unit(w, p)

            all_tes = {}

            def sct_block(qc, hp, sct, kt, c0, c1, mask0):
                # score matmuls for chunk-columns [c0, c1); if mask0, the
                # leading 128 cols are the diagonal block and get the
                # additive causal mask (0 / -1e30) folded in via I @ msk,
                # so exp emits exact zeros and no post-exp mask op exists
                for h2 in range(2):
                    hl = (2 * hp + h2) * 32
                    if mask0:
                        nc.tensor.matmul(
                            sct[:, h2, c0:c0 + 128],
                            tkf[hl:hl + 32, :, kt * 128:(kt + 1) * 128],
                            tqf[hl:hl + 32, :,
                                qc * QC + c0:qc * QC + c0 + 128],
                            start=True, stop=False, perf_mode=DR,
                            tile_position=(hl, 0))
                        nc.tensor.matmul(
                            sct[:, h2, c0:c0 + 128],
                            timat[:, :], tmsk[:, :],
                            start=False, stop=True)
                        if c0 + 128 < c1:
                            nc.tensor.matmul(
                                sct[:, h2, c0 + 128:c1],
                                tkf[hl:hl + 32, :, kt * 128:(kt + 1) * 128],
                                tqf[hl:hl + 32, :,
                                    qc * QC + c0 + 128:qc * QC + c1],
                                start=True, stop=True, perf_mode=DR,
                                tile_position=(hl, 0))
                    else:
                        nc.tensor.matmul(
                            sct[:, h2, c0:c1],
                            tkf[hl:hl + 32, :, kt * 128:(kt + 1) * 128],
                            tqf[hl:hl + 32, :,
                                qc * QC + c0:qc * QC + c1],
                            start=True, stop=True, perf_mode=DR,
                            tile_position=(hl, 0))

            def exp_block(sct, tet, c0, c1):
                nc.scalar.activation(
                    tet[:, :, c0:c1], sct[:, :, c0:c1], EXP)

            def score_unit(qc, hp, kt):
                dg = kt - 4 * qc
                coff = 128 * dg if dg > 0 else 0
                sct = scp.tile([128, 2, QC], F32, tag="sc",
                               name=f"s{qc}_{hp}_{kt}")
                tet = tep.tile([128, 2, QC], BF16, tag="te",
                               name=f"te{qc}_{hp}_{kt}")
                sct_block(qc, hp, sct, kt, coff, QC, dg >= 0)
                exp_block(sct, tet, coff, QC)
                all_tes[qc][hp][kt] = (tet, 0)

            def score_pair(qc, hp):
                # dg1 (cols 128:512) and dg3 (cols 384:512, relocated to
                # tile cols 0:128) share one psum tile and ONE exp
                # instruction, saving the 185ns/instr ACT access overhead
                kt1, kt3 = 4 * qc + 1, 4 * qc + 3
                sct = scp.tile([128, 2, QC], F32, tag="sc",
                               name=f"sp{qc}_{hp}")
                tet = tep.tile([128, 2, QC], BF16, tag="te",
                               name=f"tp{qc}_{hp}")
                for h2 in range(2):
                    hl = (2 * hp + h2) * 32
                    nc.tensor.matmul(
                        sct[:, h2, 0:128],
                        tkf[hl:hl + 32, :, kt3 * 128:(kt3 + 1) * 128],
                        tqf[hl:hl + 32, :,
                            qc * QC + 384:qc * QC + 512],
                        start=True, stop=False, perf_mode=DR,
                        tile_position=(hl, 0))
                    nc.tensor.matmul(
                        sct[:, h2, 0:128], timat[:, :], tmsk[:, :],
                        start=False, stop=True)
                    nc.tensor.matmul(
                        sct[:, h2, 128:256],
                        tkf[hl:hl + 32, :, kt1 * 128:(kt1 + 1) * 128],
                        tqf[hl:hl + 32, :,
                            qc * QC + 128:qc * QC + 256],
                        start=True, stop=False, perf_mode=DR,
                        tile_position=(hl, 0))
                    nc.tensor.matmul(
                        sct[:, h2, 128:256], timat[:, :], tmsk[:, :],
                        start=False, stop=True)
                    nc.tensor.matmul(
                        sct[:, h2, 256:512],
                        tkf[hl:hl + 32, :, kt1 * 128:(kt1 + 1) * 128],
                        tqf[hl:hl + 32, :,
                            qc * QC + 256:(qc + 1) * QC],
                        start=True, stop=True, perf_mode=DR,
                        tile_position=(hl, 0))
                exp_block(sct, tet, 0, QC)
                all_tes[qc][hp][kt1] = (tet, 0)
                all_tes[qc][hp][kt3] = (tet, 384)

            def scores_phase(qc, fuse_dg=True):
                n_kt = 4 * (qc + 1)
                all_tes[qc] = [[None] * n_kt, [None] * n_kt]
                # kt outer / hp inner: ctx for sti needs (kt<=st, BOTH hp),
                # so interleaving hp lets each sti's ctx/out-proj drain
                # during the later exps instead of after the last one.
                # Diagonal tiles dg1+dg3 are fused (score_pair) except on
                # the tail chunk, where the separate dg exps stagger the
                # per-sti ctx gates across the stream's final instructions.
                if fuse_dg:
                    for kt in range(4 * qc + 1):
                        for hp in range(2):
                            score_unit(qc, hp, kt)
                    for hp in range(2):
                        score_pair(qc, hp)
                    for hp in range(2):
                        score_unit(qc, hp, 4 * qc + 2)
                else:
                    for kt in range(n_kt):
                        for hp in range(2):
                            score_unit(qc, hp, kt)

            def ctx_phase_sti(qc, sti):
                tes = all_tes[qc]
                st = 4 * qc + sti
                cxt = cxp.tile([128, 512], F32, tag="cx",
                               name=f"cx{qc}_{sti}")
                first = None
                for hp in range(2):
                    for h2 in range(2):
                        h = 2 * hp + h2
                        for kt in range(st + 1):
                            tet, shf = tes[hp][kt]
                            m = nc.tensor.matmul(
                                cxt[:, h * 128:h * 128 + 65],
                                tet[:, h2, sti * 128 - shf:
                                    (sti + 1) * 128 - shf],
                                tv[:, kt, h, :],
                                start=(first is None), stop=False,
                                skip_group_check=True)
                            if first is None:
                                first = m
                            elif kt == 0:
                                order(m, first)
                rc = rcp.tile([128, 4], F32, tag="rc",
                              name=f"rc{qc}_{sti}")
                nc.vector.reciprocal(rc[:], apx(cxt, 64, [[128, 4], [1, 1]]))
                tcn = tcnp.tile([128, 4, 64], BF16, tag="cn",
                                name=f"cn{qc}_{sti}")
                nc.vector.tensor_mul(
                    tcn[:], apx(cxt, 0, [[128, 4], [1, 64]]),
                    colb2(rc[:], 64))
                for hp in range(2):
                    nc.sync.dma_start_transpose(
                        tct[:, hp, st * 128:(st + 1) * 128],
                        tcn[:, 2 * hp:2 * hp + 2, :])

            def ctx_phase(qc):
                for sti in range(4):
                    ctx_phase_sti(qc, sti)

            def out_proj_sti(qc, sti, tail=False):
                    st = 4 * qc + sti
                    nns = ((1, 0) if tail and cfg.get("tail_nn_swap", True)
                           else (0, 1))
                    for nn in nns:
                        if tail and nn == 1:
                            # scores psum ring is idle in the tail; borrow it
                            # so the last out-projs don't WAR-serialize on
                            # the 2-slot prp ring
                            sc_s = scp.tile([128, 2, QC], F32, tag="sc",
                                            name=f"pys{qc}_{sti}")
                            pyt = sc_s[:, 0, :]
                        else:
                            pyt = prp.tile([128, QC], F32, tag="pr",
                                           name=f"py{qc}_{sti}_{nn}")[:]
                        for hp in range(2):
                            nc.tensor.matmul(
                                pyt,
                                tct[:, hp, st * 128:(st + 1) * 128],
                                two[:, hp, nn * QC:(nn + 1) * QC],
                                start=(hp == 0), stop=(hp == 1))
                        ty = typ.tile([128, QC], BF16, tag="y",
                                      name=f"y{qc}_{sti}_{nn}")
                        # in the tail (after the last exp) ACT is idle, so
                        # split the psum->sbuf copies across ACT and DVE
                        if tail and nn == 1:
                            nc.scalar.copy(ty[:], pyt)
                        else:
                            nc.vector.tensor_copy(ty[:], pyt)
                        yeng = (nc.scalar if tail and nn == 1
                                and cfg.get("tail_y_act", True) else nc.sync)
                        yeng.dma_start(
                            y[st * 128:(st + 1) * 128,
                              nn * QC:(nn + 1) * QC], ty[:])

            def out_proj(qc):
                for sti in range(4):
                    out_proj_sti(qc, sti)

            # Emission = scheduler priority. The exp stream is the global
            # pacer: scores/exp chains first (chunk order) with the next
            # wave's q/k projection between them, then all deferrable work.
            CORDER = cfg.get("corder", [0, 1, 3, 2])
            # waves needed before a chunk's scores: all w <= qc
            emitted_qk = set()

            def need_qk(qc):
                for w_ in range(qc + 1):
                    if w_ not in emitted_qk:
                        emitted_qk.add(w_)
                        for slot in range(2):
                            qk_unit(w_, slot, "wq")
                            qk_unit(w_, slot, "wk")

            def qk_q(slot, wn, j, prt_full):
                # wave-0 quarter projection: 128-col psum group inside a
                # shared per-(wn,slot) tile, so quarters pipeline through
                # the prp ring region-wise instead of serializing on it
                fdst, ws, bias = ((tqf, WSQ, tbq) if wn == "wq"
                                  else (tkf, WSK, tbk))
                sl = slice(j * 128, (j + 1) * 128)
                prt = prt_full[:, j * 128:(j + 1) * 128]
                idx = 0
                for res, xt in COMP_QK:
                    wt = tw[("r" if res else "") + wn]
                    for ks in range(NKS):
                        nc.tensor.matmul(
                            prt,
                            wt[:, ks, :, slot * 128:(slot + 1) * 128],
                            xt[:, ks, :, sl],
                            start=(idx == 0),
                            stop=(idx == len(COMP_QK) * NKS - 1),
                            perf_mode=DR)
                        idx += 1
                nc.vector.scalar_tensor_tensor(
                    fdst[:, slot, sl], prt, 1.0 / ws,
                    colb(bias[:, slot:slot + 1], 128), op0=MUL, op1=ADD)

            def chunk0_start():
                # wave-0 projections in 128-col quarters, interleaved with
                # chunk-0 kt0's quarter-exps: the exp stream starts as soon
                # as quarter 0 of q/k is projected instead of waiting for
                # the full 512-wide wave
                emitted_qk.add(0)
                all_tes[0] = [[None] * 4, [None] * 4]
                prts = {}
                for wn in ("wq", "wk"):
                    for slot in range(2):
                        prts[wn, slot] = prp.tile(
                            [128, QC], F32, tag="pr", name=f"p0{wn}{slot}")
                scts = {}
                tets = {}
                for slot in range(2):
                    qk_q(slot, "wq", 0, prts["wq", slot])
                for slot in range(2):
                    qk_q(slot, "wk", 0, prts["wk", slot])
                hps = (0,) if cfg.get("hp1_coarse") else (0, 1)
                for hp in hps:
                    scts[hp] = scp.tile([128, 2, QC], F32, tag="sc",
                                        name=f"s0_{hp}_0")
                    tets[hp] = tep.tile([128, 2, QC], BF16, tag="te",
                                        name=f"te0_{hp}_0")
                    all_tes[0][hp][0] = (tets[hp], 0)
                    sct_block(0, hp, scts[hp], 0, 0, 128, True)
                    exp_block(scts[hp], tets[hp], 0, 128)
                for j in range(1, 4):
                    for slot in range(2):
                        qk_q(slot, "wq", j, prts["wq", slot])
                    for hp in hps:
                        sct_block(0, hp, scts[hp], 0,
                                  j * 128, (j + 1) * 128, False)
                        exp_block(scts[hp], tets[hp], j * 128, (j + 1) * 128)
                if cfg.get("hp1_coarse"):
                    score_unit(0, 1, 0)
                for slot in range(2):
                    qk_q(slot, "wk", 1, prts["wk", slot])
                for slot in range(2):
                    qk_q(slot, "wk", 3, prts["wk", slot])
                for hp in range(2):
                    score_pair(0, hp)
                for slot in range(2):
                    qk_q(slot, "wk", 2, prts["wk", slot])
                for hp in range(2):
                    score_unit(0, hp, 2)

            assert CORDER[0] == 0
            chunk0_start()
            for i, qc in enumerate(CORDER):
                if i > 0:
                    scores_phase(qc, fuse_dg=(i < NQC - 1))
                if i + 1 < NQC:
                    need_qk(CORDER[i + 1])
            emitted_v = set()

            def need_v(qc):
                for w_ in range(qc + 1):
                    if w_ not in emitted_v:
                        emitted_v.add(w_)
                        for p in range(2):
                            v_unit(w_, p)

            need_v(CORDER[0])
            for i, qc in enumerate(CORDER):
                if i + 1 < NQC:
                    need_v(CORDER[i + 1])
                if i == NQC - 1:
                    # last chunk: software-pipeline out-proj one sti behind
                    # ctx, so PE never stalls in-order on a transpose-DMA
                    # sem while later A@V work is ready behind it
                    for sti in range(4):
                        ctx_phase_sti(qc, sti)
                        if sti > 0:
                            out_proj_sti(qc, sti - 1, tail=True)
                    out_proj_sti(qc, 3, tail=True)
                else:
                    ctx_phase(qc)
                    out_proj(qc)

    nc.compile()
    return nc


def _get_nc():
    if "nc" not in _CACHE:
        _CACHE["nc"] = _build_nc()
    return _CACHE["nc"]


def make_mask():
    # additive causal mask for the diagonal 128-block: 0 where attended,
    # -1e30 where masked (exp underflows to exactly 0)
    kl = np.arange(128)[:, None]
    ql = np.arange(128)[None, :]
    return np.where(ql >= kl, 0.0, -1e30).astype(BFNP)


def _fold(t):
    # [E, N] -> [128, NKS, 2, N] with e = ks*256 + sl*128 + p
    n = t.shape[1]
    return np.ascontiguousarray(
        t.reshape(NKS, 2, 128, n).transpose(2, 0, 1, 3))


def _q8(t):
    t8 = t.astype(E4NP)
    return t8, (t - t8.astype(np.float32)).astype(E4NP)


# slot-major permutation of a head-group's 256 feature columns: the
# projection matmul then emits q/k directly in the folded layout
# (partition = head_local*32 + d_low, slot = d_high).
_PERM = np.empty(EC, np.int64)
for _slot in range(2):
    for _h in range(4):
        for _dl in range(32):
            _PERM[_slot * 128 + _h * 32 + _dl] = _h * 64 + _slot * 32 + _dl


def shard_inputs(x, Wq, bq, Wk, bk, Wv, bv, Wo, bo):
    x = np.asarray(x, dtype=np.float32)
    scale = np.float32(1.0 / np.sqrt(D))
    mask = make_mask()
    ones = np.ones((1, 64), BFNP)
    in_maps = []
    xf = []
    for b in range(B):
        x8, rx8 = _q8(_fold(np.ascontiguousarray(x[b].T)))
        xf.append((x8, rx8))
    for c in range(NCORES):
        b, g = divmod(c, 4)
        cs = slice(g * EC, (g + 1) * EC)
        wq, rwq = _q8(_fold(np.asarray(Wq[:, cs])[:, _PERM] * (scale * WSQ)))
        wk, rwk = _q8(_fold(np.asarray(Wk[:, cs])[:, _PERM] * WSK))
        wv, rwv = _q8(_fold(np.asarray(Wv[:, cs]) * WSK))
        in_maps.append({
            "x8": xf[b][0], "rx8": xf[b][1],
            "wq8": wq, "rwq8": rwq,
            "wk8": wk, "rwk8": rwk,
            "wv8": wv, "rwv8": rwv,
            "wo": np.ascontiguousarray(
                np.asarray(Wo[cs, :]).reshape(2, 128, E).transpose(1, 0, 2)
            ).astype(BFNP),
            "bq": (np.asarray(bq[cs])[_PERM] * scale).reshape(2, 128, 1).astype(np.float32),
            "bk": np.asarray(bk[cs])[_PERM].reshape(2, 128, 1).astype(np.float32),
            "bv": np.asarray(bv[cs]).reshape(1, EC).astype(np.float32),
            "msk": mask,
            "imat": np.eye(128, dtype=BFNP),
            "ones": ones,
        })
    return in_maps


def merge_y(res):
    return np.asarray(res["y"], dtype=np.float32)


def combine_outputs(results, bo):
    y = np.zeros((B, S, E), np.float32)
    for c in range(NCORES):
        b = c // 4
        y[b] += merge_y(results[c])
    y += np.asarray(bo, dtype=np.float32)[None, None, :]
    return y


def kernel(x, Wq, bq, Wk, bk, Wv, bv, Wo, bo):
    from concourse.bass_utils import run_bass_kernel_spmd

    nc = _get_nc()
    in_maps = shard_inputs(x, Wq, bq, Wk, bk, Wv, bv, Wo, bo)
    try:
        res = run_bass_kernel_spmd(nc, in_maps, core_ids=list(range(NCORES)))
    except Exception:
        # transient device errors (e.g. a wedged core) usually clear on retry
        res = run_bass_kernel_spmd(nc, in_maps, core_ids=list(range(NCORES)))
    return combine_outputs(res.results, bo)



# revision 70
# speedup vs baseline: 1.0776x; 1.0020x over previous
"""Causal self-attention (B=2, S=2048, E=1024, H=16) on 8 TRN2 NeuronCores.

Sharding: core c = 4*b + g handles batch b and head-group g (4 heads,
256 E-columns). Each core computes q/k/v projections for its head slice,
causal attention for its 4 heads, and a partial output projection
y_c = ctx_g @ Wo[rows_g]. Host sums the 4 partials per batch and adds bo.

Engine plan (CoreSim cost model driven; ~93us/core vs 172.6us naive):
  PE   - QKV projections in fp8e4 DoubleRow (K=256/instr, 0.5 cyc/row) with
         3-term error compensation (x8@W8 + x8@rW8 + rx8@W8) accumulated in
         one PSUM group; q/k projections write the scores' folded fp8
         layout (partition = head_local*32 + d_low, slot = d_high) DIRECTLY
         via slot-major host-permuted weight columns - no fold DMAs;
         scores in fp8 DoubleRow (half-rate); the diagonal 128-blocks get
         an additive 0/-1e30 causal mask accumulated into PSUM via a tiny
         I @ msk matmul, so exp emits exact zeros and there is no post-exp
         mask op on any critical chain; A@V in natural orientation
         (out = [128 q-parts, 65] bf16, the denominator rides as V's ones
         column; all 4 heads of a sti share one PSUM bank via per-byte
         zero-region semantics + a nosync order edge); out-proj in bf16
         from a transposed ctxT.
  ACT  - exp (the pacer: ~8.4M exps/core ~ 75us busy incl. 185ns/instr
         PSUM/SBUF access overhead) + startup x-DMA dispatch + the tail's
         nn=1 psum->sbuf copies and y DMAs (ACT idles after its last exp).
  DVE  - q/k bias+descale (psum->fp8), v bias+descale, denominators'
         reciprocal + ctx normalize, y psum->sbuf copies.
  Pool - small-constant SWDGE DMAs + x waves 2-3.
  SP   - weight DMAs, wave-0/1 x, XBAR dma_start_transpose (ctxT), y.

Startup: wave-0 q/k projected in 128-col quarter groups packed into one
prp tile per (tensor,slot) (disjoint-region psum groups pipeline through
the ring), x/rx wave-0 DMAed per-quarter across SP+ACT queues, and chunk-0
kt0's exps issued per-quarter - first exp at ~5.5us instead of ~14us.

Emission = Tile scheduler priority: [all scores/exp chains, chunk order
0,1,3,2, hp interleaved inside each kt so ctx/out-proj drain during the
stream; next wave's q/k proj between chunks] then [v proj + ctx + out-proj
as hole-fillers]. The last chunk software-pipelines out-proj one sti
behind ctx; its copies split ACT/DVE and its nn=1 y DMAs ride ACT.

PSUM banks (8): scores ring [128,2,512]x2 = 4 (tile per (kt,hp)),
ctx/v ring [128,512]x2 = 2, qk-proj/out-proj ring [128,512]x2 = 2 (the
tail's nn=1 out-proj borrows the then-idle scores ring).
"""

import os

import numpy as np
import ml_dtypes

os.environ.setdefault("NEURON_RT_RESET_CORES", "1")

B, S, E, H, D = 2, 2048, 1024, 16, 64
NCORES = 8
EC = 256          # E-columns per core (4 heads x 64)
QC = 512          # q-chunk width
NQC = S // QC     # 4
NKT = S // 128    # 16 k-tiles
NKS = 4           # fp8 DoubleRow K-steps (4 x 256 = 1024)

WSQ = 256.0       # fp8 range scale for Wq*(1/8)
WSK = 64.0        # fp8 range scale for Wk / Wv
E4NP = ml_dtypes.float8_e4m3
BFNP = ml_dtypes.bfloat16

_CACHE = {}


def _build_nc(cfg=None):
    cfg = cfg or {}
    import concourse.mybir as mybir
    import concourse.tile as tile
    import concourse.bass as bass
    from concourse import bacc

    F32 = mybir.dt.float32
    BF16 = mybir.dt.bfloat16
    F8 = mybir.dt.float8e4
    EXP = mybir.ActivationFunctionType.Exp
    DR = mybir.MatmulPerfMode.DoubleRow
    MUL = mybir.AluOpType.mult
    ADD = mybir.AluOpType.add

    TE_BUFS = cfg.get("te", 44)
    nc = bacc.Bacc("TRN2", target_bir_lowering=False, debug=False)

    x8 = nc.dram_tensor("x8", [128, NKS, 2, S], F8, kind="ExternalInput")
    rx8 = nc.dram_tensor("rx8", [128, NKS, 2, S], F8, kind="ExternalInput")
    wq8 = nc.dram_tensor("wq8", [128, NKS, 2, EC], F8, kind="ExternalInput")
    wk8 = nc.dram_tensor("wk8", [128, NKS, 2, EC], F8, kind="ExternalInput")
    wv8 = nc.dram_tensor("wv8", [128, NKS, 2, EC], F8, kind="ExternalInput")
    rwq8 = nc.dram_tensor("rwq8", [128, NKS, 2, EC], F8, kind="ExternalInput")
    rwk8 = nc.dram_tensor("rwk8", [128, NKS, 2, EC], F8, kind="ExternalInput")
    rwv8 = nc.dram_tensor("rwv8", [128, NKS, 2, EC], F8, kind="ExternalInput")
    wo = nc.dram_tensor("wo", [128, 2, E], BF16, kind="ExternalInput")
    bq = nc.dram_tensor("bq", [2, 128, 1], F32, kind="ExternalInput")
    bk = nc.dram_tensor("bk", [2, 128, 1], F32, kind="ExternalInput")
    bv = nc.dram_tensor("bv", [1, EC], F32, kind="ExternalInput")
    msk = nc.dram_tensor("msk", [128, 128], BF16, kind="ExternalInput")
    imat = nc.dram_tensor("imat", [128, 128], BF16, kind="ExternalInput")
    ones = nc.dram_tensor("ones", [1, 64], BF16, kind="ExternalInput")

    y = nc.dram_tensor("y", [S, E], BF16, kind="ExternalOutput")

    with tile.TileContext(nc) as tc:
        with (
            tc.tile_pool(name="weights", bufs=1) as wpool,
            tc.tile_pool(name="xp", bufs=1) as xp,
            tc.tile_pool(name="qkv", bufs=1) as qkv,
            tc.tile_pool(name="tep", bufs=TE_BUFS) as tep,
            tc.tile_pool(name="tcnp", bufs=8) as tcnp,
            tc.tile_pool(name="rcp", bufs=4) as rcp,
            tc.tile_pool(name="ctp", bufs=1) as ctp,
            tc.tile_pool(name="typ", bufs=10) as typ,
            tc.tile_pool(name="smalls", bufs=1) as smalls,
            tc.tile_pool(name="scp", bufs=2, space="PSUM") as scp,
            tc.tile_pool(name="cxp", bufs=2, space="PSUM") as cxp,
            tc.tile_pool(name="prp", bufs=2, space="PSUM") as prp,
        ):
            # ---- small constants (Pool SWDGE queue) ----
            tbq = smalls.tile([128, 2], F32, tag="bq")
            tbk = smalls.tile([128, 2], F32, tag="bk")
            tbv = smalls.tile([128, EC], F32, tag="bv")
            tmsk = smalls.tile([128, 128], BF16, tag="msk")
            timat = smalls.tile([128, 128], BF16, tag="imat")

            for r in range(2):
                nc.gpsimd.dma_start(tbq[:, r:r + 1], bq[r])
                nc.gpsimd.dma_start(tbk[:, r:r + 1], bk[r])
            bvap = bv[0, :]
            bv_b = bass.AP(tensor=bvap.tensor, offset=bvap.offset,
                           ap=[[0, 128]] + list(bvap.ap))
            nc.gpsimd.dma_start(tbv[:], bv_b)
            nc.gpsimd.dma_start(tmsk[:], msk[:])
            nc.gpsimd.dma_start(timat[:], imat[:])

            # ---- x fp8 (+ residual), loaded per 512-wide wave ----
            tx = xp.tile([128, NKS, 2, S], F8, tag="x8")
            trx = xp.tile([128, NKS, 2, S], F8, tag="rx8")

            def ldx(w):
                sl = slice(w * QC, (w + 1) * QC)
                nc.gpsimd.dma_start(tx[:, :, :, sl], x8[:, :, :, sl])
                nc.gpsimd.dma_start(trx[:, :, :, sl], rx8[:, :, :, sl])

            # wave-0 x + q/k weights first so the q/k chain starts ASAP.
            # Weights dispatch on SP, wave-0 x/rx on DVE (both idle at start)
            # so neither queue serializes the first-exp critical path.
            sl0 = slice(0, QC)
            tw = {}

            def ldw(nm, dram, eng=None):
                t = wpool.tile([128, NKS, 2, EC], F8, tag=nm, name=nm)
                (eng or nc.sync).dma_start(t[:], dram[:])
                tw[nm] = t

            # startup: wave-0 x/rx arrive in 128-col quarters, spread over
            # the SP and ACT HWDGE queues so quarter 0 + the q/k weights
            # land as early as possible for the first quarter-projection
            def ldxq(eng, xt, dram, j):
                eng.dma_start(xt[:, :, :, j * 128:(j + 1) * 128],
                              dram[:, :, :, j * 128:(j + 1) * 128])

            ldxq(nc.sync, tx, x8, 0)
            ldw("wq", wq8)
            ldxq(nc.scalar, trx, rx8, 0)
            ldw("wk", wk8)
            ldw("rwk", rwk8, eng=nc.scalar)
            ldw("rwq", rwq8)
            for j in range(1, 4):
                ldxq(nc.scalar, trx, rx8, j)
            for j in range(1, 4):
                ldxq(nc.sync, tx, x8, j)
            # wave-1 x on SP so its q/k projection is ready the moment
            # chunk-0's exps drain (Pool's SWDGE queue is too slow for it)
            sl1 = slice(QC, 2 * QC)
            nc.sync.dma_start(tx[:, :, :, sl1], x8[:, :, :, sl1])
            nc.sync.dma_start(trx[:, :, :, sl1], rx8[:, :, :, sl1])
            ldw("wv", wv8)
            ldw("rwv", rwv8)
            two = wpool.tile([128, 2, E], BF16, tag="wo")
            nc.sync.dma_start(two[:], wo[:])
            for w_ in range(2, NQC):
                ldx(w_)

            # ---- PE clock warmup ----
            # the cost model ramps PE to full clock only after 3us of
            # continuous busy; without this, the whole startup projection
            # phase runs at half clock. Chain a few dummy matmuls on a
            # zeroed scratch tile from t~0.7us so the real matmuls (~3.5us)
            # run at full speed.
            NWARM = cfg.get("warm", 0)
            if NWARM:
                tscr = smalls.tile([128, QC], BF16, tag="scr")
                nc.vector.memzero(tscr[:])
                pwt = prp.tile([128, QC], F32, tag="pr", name="pwarm")
                for _ in range(NWARM):
                    nc.tensor.matmul(pwt[:], tscr[:, 0:128], tscr[:],
                                     start=True, stop=True)

            # ---- persistent activations ----
            # folded fp8 layout: partition = head_local*32 + d_low,
            # slot = d_high half; written DIRECTLY by the projection
            # (weight columns are slot-major-permuted on the host), so no
            # partition-shift fold DMAs are needed.
            tqf = qkv.tile([128, 2, S], F8, tag="qf", name="qf")
            tkf = qkv.tile([128, 2, S], F8, tag="kf", name="kf")
            # v1: [128, kt, head, 65]; col 64 of each head block = 1.0
            tv = qkv.tile([128, NKT, 4, 65], BF16, tag="v")
            onesap = ones[0, 0:1]
            ones_v = bass.AP(tensor=onesap.tensor, offset=onesap.offset,
                             ap=[[0, 128], [0, NKT * 4], [0, 1]])
            nc.gpsimd.dma_start(tv[:, :, :, 64:65], ones_v)

            tct = ctp.tile([128, 2, S], BF16, tag="ct")  # folded ctxT

            def colb(ap1, n):
                # [128, 1] AP -> [128, n] stride-0 broadcast
                return bass.AP(tensor=ap1.tensor, offset=ap1.offset,
                               ap=[list(ap1.ap[0]), [0, n]])

            def colb2(ap2, n):
                # [128, k] AP -> [128, k, n] stride-0 broadcast
                return bass.AP(tensor=ap2.tensor, offset=ap2.offset,
                               ap=[list(ap2.ap[0]), list(ap2.ap[1]), [0, n]])

            def apx(t, off, dims):
                # partition dim of tile t + custom free dims at f32-col offset
                a = t[:]
                return bass.AP(tensor=a.tensor, offset=a.offset + off,
                               ap=[list(a.ap[0])] + [list(d) for d in dims])

            def order(later, first):
                bass._add_dep_helper(later.ins, first.ins, sync=False,
                                     reason="psum zero-region order")

            COMP = ((None, tx), ("r", tx), (None, trx))  # (w-residual?, x-tensor)
            COMP_QK = COMP[:2] if cfg.get("qk2") else COMP
            COMP_V = COMP[:2] if cfg.get("v2") else COMP

            def qk_unit(w, slot, wn, q4=None):
                fdst, ws, bias = ((tqf, WSQ, tbq) if wn == "wq"
                                  else (tkf, WSK, tbk))
                if q4 is None:
                    sl = slice(w * QC, (w + 1) * QC)
                    width = QC
                else:
                    sl = slice(w * QC + q4 * 128, w * QC + (q4 + 1) * 128)
                    width = 128
                prt = prp.tile([128, QC], F32, tag="pr",
                               name=f"p{wn}{w}_{slot}_{q4}")[:, 0:width]
                idx = 0
                for res, xt in COMP_QK:
                    wt = tw[("r" if res else "") + wn]
                    for ks in range(NKS):
                        nc.tensor.matmul(
                            prt,
                            wt[:, ks, :, slot * 128:(slot + 1) * 128],
                            xt[:, ks, :, sl],
                            start=(idx == 0),
                            stop=(idx == len(COMP_QK) * NKS - 1),
                            perf_mode=DR)
                        idx += 1
                nc.vector.scalar_tensor_tensor(
                    fdst[:, slot, sl], prt, 1.0 / ws,
                    colb(bias[:, slot:slot + 1], width), op0=MUL, op1=ADD)

            def v_unit(w, pair):
                cxt = cxp.tile([128, 512], F32, tag="cx", name=f"pv{w}_{pair}")
                first = None
                for st2 in range(2):
                    st = 4 * w + 2 * pair + st2
                    idx = 0
                    for res, xt in COMP_V:
                        wt = tw[("r" if res else "") + "wv"]
                        for ks in range(NKS):
                            m = nc.tensor.matmul(
                                cxt[:, st2 * EC:(st2 + 1) * EC],
                                xt[:, ks, :, st * 128:(st + 1) * 128],
                                wt[:, ks, :, :],
                                start=(first is None), stop=False,
                                perf_mode=DR, skip_group_check=True)
                            if first is None:
                                first = m
                            elif idx == 0:
                                order(m, first)
                            idx += 1
                for st2 in range(2):
                    st = 4 * w + 2 * pair + st2
                    nc.vector.scalar_tensor_tensor(
                        tv[:, st, :, 0:64],
                        apx(cxt, st2 * EC, [[64, 4], [1, 64]]),
                        1.0 / WSK,
                        tbv[:].rearrange("p (h d) -> p h d", h=4),
                        op0=MUL, op1=ADD)

            def proj_wave(w):
                for slot in range(2):
                    qk_unit(w, slot, "wq")
                    qk_unit(w, slot, "wk")
                for p in range(2):
                    v_unit(w, p)

            all_tes = {}

            def sct_block(qc, hp, sct, kt, c0, c1, mask0):
                # score matmuls for chunk-columns [c0, c1); if mask0, the
                # leading 128 cols are the diagonal block and get the
                # additive causal mask (0 / -1e30) folded in via I @ msk,
                # so exp emits exact zeros and no post-exp mask op exists
                for h2 in range(2):
                    hl = (2 * hp + h2) * 32
                    if mask0:
                        nc.tensor.matmul(
                            sct[:, h2, c0:c0 + 128],
                            tkf[hl:hl + 32, :, kt * 128:(kt + 1) * 128],
                            tqf[hl:hl + 32, :,
                                qc * QC + c0:qc * QC + c0 + 128],
                            start=True, stop=False, perf_mode=DR,
                            tile_position=(hl, 0))
                        nc.tensor.matmul(
                            sct[:, h2, c0:c0 + 128],
                            timat[:, :], tmsk[:, :],
                            start=False, stop=True)
                        if c0 + 128 < c1:
                            nc.tensor.matmul(
                                sct[:, h2, c0 + 128:c1],
                                tkf[hl:hl + 32, :, kt * 128:(kt + 1) * 128],
                                tqf[hl:hl + 32, :,
                                    qc * QC + c0 + 128:qc * QC + c1],
                                start=True, stop=True, perf_mode=DR,
                                tile_position=(hl, 0))
                    else:
                        nc.tensor.matmul(
                            sct[:, h2, c0:c1],
                            tkf[hl:hl + 32, :, kt * 128:(kt + 1) * 128],
                            tqf[hl:hl + 32, :,
                                qc * QC + c0:qc * QC + c1],
                            start=True, stop=True, perf_mode=DR,
                            tile_position=(hl, 0))

            def exp_block(sct, tet, c0, c1):
                nc.scalar.activation(
                    tet[:, :, c0:c1], sct[:, :, c0:c1], EXP)

            def score_unit(qc, hp, kt):
                dg = kt - 4 * qc
                coff = 128 * dg if dg > 0 else 0
                sct = scp.tile([128, 2, QC], F32, tag="sc",
                               name=f"s{qc}_{hp}_{kt}")
                tet = tep.tile([128, 2, QC], BF16, tag="te",
                               name=f"te{qc}_{hp}_{kt}")
                sct_block(qc, hp, sct, kt, coff, QC, dg >= 0)
                exp_block(sct, tet, coff, QC)
                all_tes[qc][hp][kt] = (tet, 0)

            def score_pair(qc, hp):
                # dg1 (cols 128:512) and dg3 (cols 384:512, relocated to
                # tile cols 0:128) share one psum tile and ONE exp
                # instruction, saving the 185ns/instr ACT access overhead
                kt1, kt3 = 4 * qc + 1, 4 * qc + 3
                sct = scp.tile([128, 2, QC], F32, tag="sc",
                               name=f"sp{qc}_{hp}")
                tet = tep.tile([128, 2, QC], BF16, tag="te",
                               name=f"tp{qc}_{hp}")
                for h2 in range(2):
                    hl = (2 * hp + h2) * 32
                    nc.tensor.matmul(
                        sct[:, h2, 0:128],
                        tkf[hl:hl + 32, :, kt3 * 128:(kt3 + 1) * 128],
                        tqf[hl:hl + 32, :,
                            qc * QC + 384:qc * QC + 512],
                        start=True, stop=False, perf_mode=DR,
                        tile_position=(hl, 0))
                    nc.tensor.matmul(
                        sct[:, h2, 0:128], timat[:, :], tmsk[:, :],
                        start=False, stop=True)
                    nc.tensor.matmul(
                        sct[:, h2, 128:256],
                        tkf[hl:hl + 32, :, kt1 * 128:(kt1 + 1) * 128],
                        tqf[hl:hl + 32, :,
                            qc * QC + 128:qc * QC + 256],
                        start=True, stop=False, perf_mode=DR,
                        tile_position=(hl, 0))
                    nc.tensor.matmul(
                        sct[:, h2, 128:256], timat[:, :], tmsk[:, :],
                        start=False, stop=True)
                    nc.tensor.matmul(
                        sct[:, h2, 256:512],
                        tkf[hl:hl + 32, :, kt1 * 128:(kt1 + 1) * 128],
                        tqf[hl:hl + 32, :,
                            qc * QC + 256:(qc + 1) * QC],
                        start=True, stop=True, perf_mode=DR,
                        tile_position=(hl, 0))
                exp_block(sct, tet, 0, QC)
                all_tes[qc][hp][kt1] = (tet, 0)
                all_tes[qc][hp][kt3] = (tet, 384)

            def score_dg2_hpfused(qc):
                # dg2 uses cols [256:512] only; pack hp0 there and hp1 at
                # [0:256] in ONE tile with ONE exp for both head-pairs
                kt2 = 4 * qc + 2
                sct = scp.tile([128, 2, QC], F32, tag="sc",
                               name=f"s2f{qc}")
                tet = tep.tile([128, 2, QC], BF16, tag="te",
                               name=f"t2f{qc}")
                for hp in range(2):
                    base = 256 if hp == 0 else 0
                    for h2 in range(2):
                        hl = (2 * hp + h2) * 32
                        nc.tensor.matmul(
                            sct[:, h2, base:base + 128],
                            tkf[hl:hl + 32, :, kt2 * 128:(kt2 + 1) * 128],
                            tqf[hl:hl + 32, :,
                                qc * QC + 256:qc * QC + 384],
                            start=True, stop=False, perf_mode=DR,
                            tile_position=(hl, 0))
                        nc.tensor.matmul(
                            sct[:, h2, base:base + 128],
                            timat[:, :], tmsk[:, :],
                            start=False, stop=True)
                        nc.tensor.matmul(
                            sct[:, h2, base + 128:base + 256],
                            tkf[hl:hl + 32, :, kt2 * 128:(kt2 + 1) * 128],
                            tqf[hl:hl + 32, :,
                                qc * QC + 384:(qc + 1) * QC],
                            start=True, stop=True, perf_mode=DR,
                            tile_position=(hl, 0))
                exp_block(sct, tet, 0, QC)
                all_tes[qc][0][kt2] = (tet, 0)
                all_tes[qc][1][kt2] = (tet, 256)

            def score_dg3_hpfused(qc):
                # dg3 is 128 wide; hp0 at cols [0:128], hp1 at [128:256],
                # one exp for both
                kt3 = 4 * qc + 3
                sct = scp.tile([128, 2, QC], F32, tag="sc",
                               name=f"s3f{qc}")
                tet = tep.tile([128, 2, QC], BF16, tag="te",
                               name=f"t3f{qc}")
                for hp in range(2):
                    for h2 in range(2):
                        hl = (2 * hp + h2) * 32
                        nc.tensor.matmul(
                            sct[:, h2, hp * 128:(hp + 1) * 128],
                            tkf[hl:hl + 32, :, kt3 * 128:(kt3 + 1) * 128],
                            tqf[hl:hl + 32, :,
                                qc * QC + 384:(qc + 1) * QC],
                            start=True, stop=False, perf_mode=DR,
                            tile_position=(hl, 0))
                        nc.tensor.matmul(
                            sct[:, h2, hp * 128:(hp + 1) * 128],
                            timat[:, :], tmsk[:, :],
                            start=False, stop=True)
                exp_block(sct, tet, 0, 256)
                all_tes[qc][0][kt3] = (tet, 384)
                all_tes[qc][1][kt3] = (tet, 256)

            def scores_phase(qc, fuse_dg=True):
                n_kt = 4 * (qc + 1)
                all_tes[qc] = [[None] * n_kt, [None] * n_kt]
                # kt outer / hp inner: ctx for sti needs (kt<=st, BOTH hp),
                # so interleaving hp lets each sti's ctx/out-proj drain
                # during the later exps instead of after the last one.
                # Diagonal tiles dg1+dg3 are fused (score_pair) except on
                # the tail chunk, where the separate dg exps stagger the
                # per-sti ctx gates across the stream's final instructions.
                if fuse_dg:
                    for kt in range(4 * qc + 1):
                        for hp in range(2):
                            score_unit(qc, hp, kt)
                    for hp in range(2):
                        score_pair(qc, hp)
                    score_dg2_hpfused(qc)
                else:
                    # tail chunk: keep per-sti ctx gates staggered across
                    # the last exps (dg1 per hp, then dg2, then dg3)
                    for kt in range(4 * qc + 2):
                        for hp in range(2):
                            score_unit(qc, hp, kt)
                    score_dg2_hpfused(qc)
                    score_dg3_hpfused(qc)

            def ctx_phase_sti(qc, sti, tail=False):
                tes = all_tes[qc]
                st = 4 * qc + sti
                cxt = cxp.tile([128, 512], F32, tag="cx",
                               name=f"cx{qc}_{sti}")
                first = None
                for hp in range(2):
                    for h2 in range(2):
                        h = 2 * hp + h2
                        for kt in range(st + 1):
                            tet, shf = tes[hp][kt]
                            m = nc.tensor.matmul(
                                cxt[:, h * 128:h * 128 + 65],
                                tet[:, h2, sti * 128 - shf:
                                    (sti + 1) * 128 - shf],
                                tv[:, kt, h, :],
                                start=(first is None), stop=False,
                                skip_group_check=True)
                            if first is None:
                                first = m
                            elif kt == 0:
                                order(m, first)
                rc = rcp.tile([128, 4], F32, tag="rc",
                              name=f"rc{qc}_{sti}")
                nc.vector.reciprocal(rc[:], apx(cxt, 64, [[128, 4], [1, 1]]))
                tcn = tcnp.tile([128, 4, 64], BF16, tag="cn",
                                name=f"cn{qc}_{sti}")
                nc.vector.tensor_mul(
                    tcn[:], apx(cxt, 0, [[128, 4], [1, 64]]),
                    colb2(rc[:], 64))
                teng = (nc.scalar if tail and cfg.get("tail_tr_act")
                        else nc.sync)
                for hp in range(2):
                    teng.dma_start_transpose(
                        tct[:, hp, st * 128:(st + 1) * 128],
                        tcn[:, 2 * hp:2 * hp + 2, :])

            def ctx_phase(qc):
                for sti in range(4):
                    ctx_phase_sti(qc, sti)

            def out_proj_sti(qc, sti, tail=False):
                    st = 4 * qc + sti
                    nns = ((1, 0) if tail and cfg.get("tail_nn_swap", True)
                           else (0, 1))
                    for nn in nns:
                        if tail and nn == 1:
                            # scores psum ring is idle in the tail; borrow it
                            # so the last out-projs don't WAR-serialize on
                            # the 2-slot prp ring
                            sc_s = scp.tile([128, 2, QC], F32, tag="sc",
                                            name=f"pys{qc}_{sti}")
                            pyt = sc_s[:, 0, :]
                        else:
                            pyt = prp.tile([128, QC], F32, tag="pr",
                                           name=f"py{qc}_{sti}_{nn}")[:]
                        for hp in range(2):
                            nc.tensor.matmul(
                                pyt,
                                tct[:, hp, st * 128:(st + 1) * 128],
                                two[:, hp, nn * QC:(nn + 1) * QC],
                                start=(hp == 0), stop=(hp == 1))
                        ty = typ.tile([128, QC], BF16, tag="y",
                                      name=f"y{qc}_{sti}_{nn}")
                        # in the tail (after the last exp) ACT is idle, so
                        # split the psum->sbuf copies across ACT and DVE
                        if tail and nn == 1:
                            nc.scalar.copy(ty[:], pyt)
                        else:
                            nc.vector.tensor_copy(ty[:], pyt)
                        yeng = (nc.scalar if tail and nn == 1
                                and cfg.get("tail_y_act", True) else nc.sync)
                        yeng.dma_start(
                            y[st * 128:(st + 1) * 128,
                              nn * QC:(nn + 1) * QC], ty[:])

            def out_proj(qc):
                for sti in range(4):
                    out_proj_sti(qc, sti)

            # Emission = scheduler priority. The exp stream is the global
            # pacer: scores/exp chains first (chunk order) with the next
            # wave's q/k projection between them, then all deferrable work.
            CORDER = cfg.get("corder", [0, 1, 3, 2])
            # waves needed before a chunk's scores: all w <= qc
            emitted_qk = set()

            def need_qk(qc):
                # q units first: the next chunk's early exps (kt 0..3) need
                # only the new q columns - its k tiles are from old waves
                for w_ in range(qc + 1):
                    if w_ not in emitted_qk:
                        emitted_qk.add(w_)
                        for slot in range(2):
                            qk_unit(w_, slot, "wq")
                        for slot in range(2):
                            qk_unit(w_, slot, "wk")

            def qk_q(slot, wn, j, prt_full):
                # wave-0 quarter projection: 128-col psum group inside a
                # shared per-(wn,slot) tile, so quarters pipeline through
                # the prp ring region-wise instead of serializing on it
                fdst, ws, bias = ((tqf, WSQ, tbq) if wn == "wq"
                                  else (tkf, WSK, tbk))
                sl = slice(j * 128, (j + 1) * 128)
                prt = prt_full[:, j * 128:(j + 1) * 128]
                idx = 0
                for res, xt in COMP_QK:
                    wt = tw[("r" if res else "") + wn]
                    for ks in range(NKS):
                        nc.tensor.matmul(
                            prt,
                            wt[:, ks, :, slot * 128:(slot + 1) * 128],
                            xt[:, ks, :, sl],
                            start=(idx == 0),
                            stop=(idx == len(COMP_QK) * NKS - 1),
                            perf_mode=DR)
                        idx += 1
                nc.vector.scalar_tensor_tensor(
                    fdst[:, slot, sl], prt, 1.0 / ws,
                    colb(bias[:, slot:slot + 1], 128), op0=MUL, op1=ADD)

            def chunk0_start():
                # wave-0 projections in 128-col quarters, interleaved with
                # chunk-0 kt0's quarter-exps: the exp stream starts as soon
                # as quarter 0 of q/k is projected instead of waiting for
                # the full 512-wide wave
                emitted_qk.add(0)
                all_tes[0] = [[None] * 4, [None] * 4]
                prts = {}
                for wn in ("wq", "wk"):
                    for slot in range(2):
                        prts[wn, slot] = prp.tile(
                            [128, QC], F32, tag="pr", name=f"p0{wn}{slot}")
                scts = {}
                tets = {}
                for slot in range(2):
                    qk_q(slot, "wq", 0, prts["wq", slot])
                for slot in range(2):
                    qk_q(slot, "wk", 0, prts["wk", slot])
                hps = (0,) if cfg.get("hp1_coarse") else (0, 1)
                for hp in hps:
                    scts[hp] = scp.tile([128, 2, QC], F32, tag="sc",
                                        name=f"s0_{hp}_0")
                    tets[hp] = tep.tile([128, 2, QC], BF16, tag="te",
                                        name=f"te0_{hp}_0")
                    all_tes[0][hp][0] = (tets[hp], 0)
                    sct_block(0, hp, scts[hp], 0, 0, 128, True)
                    exp_block(scts[hp], tets[hp], 0, 128)
                for j in range(1, 4):
                    for slot in range(2):
                        qk_q(slot, "wq", j, prts["wq", slot])
                    for hp in hps:
                        sct_block(0, hp, scts[hp], 0,
                                  j * 128, (j + 1) * 128, False)
                        exp_block(scts[hp], tets[hp], j * 128, (j + 1) * 128)
                if cfg.get("hp1_coarse"):
                    score_unit(0, 1, 0)
                for slot in range(2):
                    qk_q(slot, "wk", 1, prts["wk", slot])
                for slot in range(2):
                    qk_q(slot, "wk", 3, prts["wk", slot])
                for hp in range(2):
                    score_pair(0, hp)
                for slot in range(2):
                    qk_q(slot, "wk", 2, prts["wk", slot])
                score_dg2_hpfused(0)

            assert CORDER[0] == 0
            chunk0_start()
            for i, qc in enumerate(CORDER):
                if i > 0:
                    scores_phase(qc, fuse_dg=(i < NQC - 1))
                if i + 1 < NQC:
                    need_qk(CORDER[i + 1])
            emitted_v = set()

            def need_v(qc):
                for w_ in range(qc + 1):
                    if w_ not in emitted_v:
                        emitted_v.add(w_)
                        for p in range(2):
                            v_unit(w_, p)

            need_v(CORDER[0])
            for i, qc in enumerate(CORDER):
                if i + 1 < NQC:
                    need_v(CORDER[i + 1])
                if i == NQC - 1:
                    # last chunk: software-pipeline out-proj one sti behind
                    # ctx, so PE never stalls in-order on a transpose-DMA
                    # sem while later A@V work is ready behind it
                    for sti in range(4):
                        ctx_phase_sti(qc, sti, tail=True)
                        if sti > 0:
                            out_proj_sti(qc, sti - 1, tail=True)
                    out_proj_sti(qc, 3, tail=True)
                else:
                    ctx_phase(qc)
                    out_proj(qc)

    nc.compile()
    return nc


def _get_nc():
    if "nc" not in _CACHE:
        _CACHE["nc"] = _build_nc()
    return _CACHE["nc"]


def make_mask():
    # additive causal mask for the diagonal 128-block: 0 where attended,
    # -1e30 where masked (exp underflows to exactly 0)
    kl = np.arange(128)[:, None]
    ql = np.arange(128)[None, :]
    return np.where(ql >= kl, 0.0, -1e30).astype(BFNP)


def _fold(t):
    # [E, N] -> [128, NKS, 2, N] with e = ks*256 + sl*128 + p
    n = t.shape[1]
    return np.ascontiguousarray(
        t.reshape(NKS, 2, 128, n).transpose(2, 0, 1, 3))


def _q8(t):
    t8 = t.astype(E4NP)
    return t8, (t - t8.astype(np.float32)).astype(E4NP)


# slot-major permutation of a head-group's 256 feature columns: the
# projection matmul then emits q/k directly in the folded layout
# (partition = head_local*32 + d_low, slot = d_high).
_PERM = np.empty(EC, np.int64)
for _slot in range(2):
    for _h in range(4):
        for _dl in range(32):
            _PERM[_slot * 128 + _h * 32 + _dl] = _h * 64 + _slot * 32 + _dl


def shard_inputs(x, Wq, bq, Wk, bk, Wv, bv, Wo, bo):
    x = np.asarray(x, dtype=np.float32)
    scale = np.float32(1.0 / np.sqrt(D))
    mask = make_mask()
    ones = np.ones((1, 64), BFNP)
    in_maps = []
    xf = []
    for b in range(B):
        x8, rx8 = _q8(_fold(np.ascontiguousarray(x[b].T)))
        xf.append((x8, rx8))
    for c in range(NCORES):
        b, g = divmod(c, 4)
        cs = slice(g * EC, (g + 1) * EC)
        wq, rwq = _q8(_fold(np.asarray(Wq[:, cs])[:, _PERM] * (scale * WSQ)))
        wk, rwk = _q8(_fold(np.asarray(Wk[:, cs])[:, _PERM] * WSK))
        wv, rwv = _q8(_fold(np.asarray(Wv[:, cs]) * WSK))
        in_maps.append({
            "x8": xf[b][0], "rx8": xf[b][1],
            "wq8": wq, "rwq8": rwq,
            "wk8": wk, "rwk8": rwk,
            "wv8": wv, "rwv8": rwv,
            "wo": np.ascontiguousarray(
                np.asarray(Wo[cs, :]).reshape(2, 128, E).transpose(1, 0, 2)
            ).astype(BFNP),
            "bq": (np.asarray(bq[cs])[_PERM] * scale).reshape(2, 128, 1).astype(np.float32),
            "bk": np.asarray(bk[cs])[_PERM].reshape(2, 128, 1).astype(np.float32),
            "bv": np.asarray(bv[cs]).reshape(1, EC).astype(np.float32),
            "msk": mask,
            "imat": np.eye(128, dtype=BFNP),
            "ones": ones,
        })
    return in_maps


def merge_y(res):
    return np.asarray(res["y"], dtype=np.float32)


def combine_outputs(results, bo):
    y = np.zeros((B, S, E), np.float32)
    for c in range(NCORES):
        b = c // 4
        y[b] += merge_y(results[c])
    y += np.asarray(bo, dtype=np.float32)[None, None, :]
    return y


def kernel(x, Wq, bq, Wk, bk, Wv, bv, Wo, bo):
    from concourse.bass_utils import run_bass_kernel_spmd

    nc = _get_nc()
    in_maps = shard_inputs(x, Wq, bq, Wk, bk, Wv, bv, Wo, bo)
    try:
        res = run_bass_kernel_spmd(nc, in_maps, core_ids=list(range(NCORES)))
    except Exception:
        # transient device errors (e.g. a wedged core) usually clear on retry
        res = run_bass_kernel_spmd(nc, in_maps, core_ids=list(range(NCORES)))
    return combine_outputs(res.results, bo)

